# revision 18
# baseline (speedup 1.0000x reference)
"""AdaLN transformer block on 8 TRN2 NeuronCores (Bass/Tile), fp8 edition.

Sharding: 4096 tokens (B*S) split 8 ways -> 512 tokens/core; cores (2b, 2b+1)
own batch element b. Attention needs full-sequence K/V per batch element:
pairwise AllGather (replica groups [[0,1],[2,3],[4,5],[6,7]]) of fp8 K^T /
V(+1/16 col).

Precision: all projections run in fp8 e4m3 with DoubleRow perf mode (two
128-row contraction chunks per matmul -> 2x PE throughput; weights are
host-prescaled x32 and pair-packed [128,2,N]). Scores use e4m3 qT/kT with the
softmax exp prescale A5=4/ln2 folded into the q eviction; probabilities are
e5m2 via ScalarE exact exp (even key tiles) and a DVE int8 Schraudolph trick
(odd key tiles, round-to-nearest, tuned B5). PV runs fp8 DoubleRow over
key-tile pairs; the V ones-column is 1/16 so the reciprocal fold lands oT in
e4m3 range (x16). The f32 spine (x residual, LN stats, PSUM accumulation,
softmax denominators) keeps total rel err ~1e-2 (tolerance 2e-2).
"""
import os
import math
import numpy as np

import concourse.bass as bass
import concourse.bacc as bacc
import concourse.tile as tile
import concourse.mybir as mybir
from concourse import bass_utils

B, S, H, NH = 4, 1024, 1024, 16
DH = H // NH               # 64
EPS = 1e-5
NC = 8
T = (B * S) // NC          # 512 tokens per core
TC = T // 128              # 4
HC = H // 128              # 8
HP = HC // 2               # 4 pair tiles per H-contraction

VA_G = 4 * (DH + 1)        # 260: one 4-head group's v_aug row chunk (fp8)
VA_ROW = 4 * VA_G          # 1040: full v_aug row (16 heads)
VA_P = 272                 # padded pair-slot stride in vt tiles (mult of 16)
KT_CH = 128 * T // 2       # 32768: one [128,T] fp8 kT chunk in bf16 elems
KT_LEN = HC * KT_CH        # 262144
VA_LEN = T * VA_ROW // 2   # 266240
RANK_LEN = KT_LEN + VA_LEN
HALF_KT = 4 * KT_CH
HALF_VA = T * (VA_ROW // 2) // 2
HALF_LEN = HALF_KT + HALF_VA

F32 = mybir.dt.float32
BF16 = mybir.dt.bfloat16
E4 = mybir.dt.float8e4
E5 = mybir.dt.float8e5
I8 = mybir.dt.int8
I32 = mybir.dt.int32
AF = mybir.ActivationFunctionType
OP = mybir.AluOpType
DR = mybir.MatmulPerfMode.DoubleRow

WS = 32.0                  # host weight prescale (power of 2)
A5 = 4.0 / math.log(2.0)   # e5m2 Schraudolph scale (folded into q eviction)
B5 = 59.76                 # tuned for round-to-nearest f32->int8
ONE_C = 1.0 / 16.0         # v_aug ones column; recip fold scales oT x16

W_NAMES = ["Wsq", "Wsk", "Wsv", "Wso", "Wcq", "Wck", "Wcv", "Wco",
           "ffn_w1", "ffn_w2"]

LAST_RESULT = None
_BUILD_CACHE = {}


def _build():
    nc = bacc.Bacc("TRN2", target_bir_lowering=False, debug=False,
                   num_devices=NC)
    ext = {}
    for nm in ["x", "h", "t"]:
        ext[nm] = nc.dram_tensor(nm, [T, H], BF16, kind="ExternalInput")
    ext["em"] = nc.dram_tensor("em", [1, T], I32, kind="ExternalInput")
    ext["mk"] = nc.dram_tensor("mk", [1, T], I32, kind="ExternalInput")
    for nm in W_NAMES:
        ext[nm] = nc.dram_tensor(nm, [4 * 128, 2 * H], E4, kind="ExternalInput")
    ext["ada_w"] = nc.dram_tensor("ada_w", [4 * 128, 2 * 2 * H], E4,
                                  kind="ExternalInput")
    ext["c_ones"] = nc.dram_tensor("c_ones", [1, T], BF16, kind="ExternalInput")
    ext["c_ident"] = nc.dram_tensor("c_ident", [128, 128], F32, kind="ExternalInput")
    ext["c_identb"] = nc.dram_tensor("c_identb", [128, 128], BF16, kind="ExternalInput")
    out_ext = nc.dram_tensor("out", [T, H], F32, kind="ExternalOutput")

    with tile.TileContext(nc) as tc:
        _emit(nc, tc, ext, out_ext)
    nc.compile()
    return nc


def _emit(nc, tc, ext, out_ext):
    import contextlib
    ctx = contextlib.ExitStack()
    with ctx:
        full = ctx.enter_context(tc.tile_pool(name="full", bufs=13))
        halfp = ctx.enter_context(tc.tile_pool(name="halfp", bufs=16))
        e4p = ctx.enter_context(tc.tile_pool(name="e4p", bufs=14))
        wpool = ctx.enter_context(tc.tile_pool(name="wpool", bufs=16))
        bmod = ctx.enter_context(tc.tile_pool(name="bmod", bufs=8))
        ktp = ctx.enter_context(tc.tile_pool(name="ktp", bufs=3))
        vtp = ctx.enter_context(tc.tile_pool(name="vtp", bufs=8))
        ppp = ctx.enter_context(tc.tile_pool(name="ppp", bufs=3))
        bcsp = ctx.enter_context(tc.tile_pool(name="bcsp", bufs=2))
        vaugp = ctx.enter_context(tc.tile_pool(name="vaugp", bufs=4))
        smalls = ctx.enter_context(tc.tile_pool(name="smalls", bufs=1))
        stat = ctx.enter_context(tc.tile_pool(name="stat", bufs=8))
        rowp = ctx.enter_context(tc.tile_pool(name="rowp", bufs=2))
        ps = ctx.enter_context(tc.tile_pool(name="ps", bufs=4, space="PSUM"))
        pspair = ctx.enter_context(tc.tile_pool(name="pspair", bufs=2, space="PSUM"))
        dram = ctx.enter_context(tc.tile_pool(name="dram", bufs=1, space="DRAM"))

        # ---------------- constants ----------------
        ones = smalls.tile([1, T], BF16, name="ones", tag="ones")
        nc.sync.dma_start(ones[:], ext["c_ones"].ap())
        ident = smalls.tile([128, 128], F32, name="ident", tag="ident")
        nc.sync.dma_start(ident[:], ext["c_ident"].ap())
        identb = smalls.tile([128, 128], BF16, name="identb", tag="identb")
        nc.sync.dma_start(identb[:], ext["c_identb"].ap())
        eps_t = smalls.tile([128, 1], F32, name="eps_t", tag="eps_t")
        nc.vector.memset(eps_t[:], EPS)
        # HAM warmup: keep PE busy during the initial input DMA.
        wu = ps.tile([128, 512], F32, name="wu", tag="ps")
        wu_src = ident[:].bitcast(BF16)
        for _ in range(30):
            nc.tensor.matmul(wu[:, 0:256], wu_src[:, 0:128], wu_src[:],
                             start=True, stop=True)

        # skew absorber: tiny AllGather at t=0; DVE syncs on it so all cores
        # align before the heavy phases (later collectives then see ~0 skew).
        dummy_in = dram.tile([128], BF16, name="dummy_in", tag="dummy_in")
        dummy_out = dram.tile([256], BF16, name="dummy_out", tag="dummy_out")
        dsb = smalls.tile([1, 128], BF16, name="dsb", tag="dsb")
        nc.vector.memset(dsb[:], 0.0)
        nc.sync.dma_start(dummy_in.rearrange("(p f) -> p f", p=1), dsb[:])
        nc.gpsimd.collective_compute(
            "AllGather", OP.bypass,
            replica_groups=[[0, 1], [2, 3], [4, 5], [6, 7]],
            ins=[dummy_in.opt()], outs=[dummy_out.opt()])
        dsb2 = smalls.tile([1, 256], BF16, name="dsb2", tag="dsb2")
        nc.sync.dma_start(dsb2[:], dummy_out.rearrange("(p f) -> p f", p=1))
        dsb3 = smalls.tile([1, 256], BF16, name="dsb3", tag="dsb3")
        nc.vector.tensor_copy(dsb3[:], dsb2[:])

        # ---------------- input loads (h first: needed earliest) ----------
        h_sb, t_sb, x_sb = [], [], []
        for mt in range(TC):
            th = halfp.tile([128, H], BF16, name="h", tag="bighalf")
            nc.sync.dma_start(th[:], ext["h"].ap()[mt * 128:(mt + 1) * 128, :])
            h_sb.append(th)

        def load_weight(nm, half=None):
            """fp8 pair-packed weight: 4 tiles [128, 2, N] (rearranged APs)."""
            n = 2 * H if nm == "ada_w" else H
            if half is not None:
                n = n // 2
            tiles = []
            for j in range(4):
                t_ = wpool.tile([128, 2 * n], E4, name="w_" + nm, tag="w")
                if half is not None:
                    t3 = t_[:].rearrange("p (two f) -> p two f", two=2)
                    for s in range(2):
                        nc.sync.dma_start(
                            t3[:, s, :],
                            ext[nm].ap()[j * 128:(j + 1) * 128,
                                         s * 2 * H + half * n:
                                         s * 2 * H + (half + 1) * n])
                else:
                    nc.sync.dma_start(t_[:], ext[nm].ap()[j * 128:(j + 1) * 128, :])
                tiles.append(t_[:].rearrange("p (two f) -> p two f", two=2))
            return tiles

        for mt in range(TC):
            tt = halfp.tile([128, H], BF16, name="tin", tag="bighalf")
            nc.sync.dma_start(tt[:], ext["t"].ap()[mt * 128:(mt + 1) * 128, :])
            t_sb.append(tt)
        for mt in range(TC):
            tx = halfp.tile([128, H], BF16, name="x", tag="bighalf")
            nc.sync.dma_start(tx[:], ext["x"].ap()[mt * 128:(mt + 1) * 128, :])
            x_sb.append(tx)
        ada_tiles0 = load_weight("ada_w", half=0)

        def mask_bcast(name, tagn, scale):
            """[128, 2T] bf16 broadcast of mask*scale (same mask both halves)."""
            mi = smalls.tile([1, T], I32, name=tagn + "_i", tag=tagn + "_i")
            nc.sync.dma_start(mi[:], ext[name].ap())
            mf = smalls.tile([1, T], F32, name=tagn + "_f", tag=tagn + "_f")
            nc.vector.tensor_copy(mf[:], mi[:])
            mr = smalls.tile([1, T], BF16, name=tagn + "_r", tag=tagn + "_r")
            nc.vector.tensor_scalar_mul(mr[:], mf[:], scale)
            bc = smalls.tile([128, 2 * T], BF16, name=tagn + "_bc", tag=tagn + "_bc")
            p = pspair.tile([128, 1024], F32, name="spair", tag="sp")
            nc.tensor.matmul(p[:, 0:512], ones[:, 0:128], mr[:], start=True, stop=True)
            nc.tensor.matmul(p[:, 512:1024], ones[:, 0:128], mr[:], start=True, stop=True)
            nc.vector.tensor_copy(bc[:], p[:])
            return bc

        ag_in_s0 = dram.tile([HALF_LEN], BF16, name="agins0", tag="agins0")
        ag_out_s0 = dram.tile([2 * HALF_LEN], BF16, name="agouts0", tag="agouts0")
        ag_in_s1 = dram.tile([HALF_LEN], BF16, name="agins1", tag="agins1")
        ag_out_s1 = dram.tile([2 * HALF_LEN], BF16, name="agouts1", tag="agouts1")
        ag_in_c = dram.tile([RANK_LEN], BF16, name="aginc", tag="aginc")
        ag_out_c = dram.tile([2 * RANK_LEN], BF16, name="agoutc", tag="agoutc")

        # ---------------- helpers ----------------
        def layernorm_tile(src, out_dt, out_tag, apply_engine, out_pool=None):
            st = stat.tile([128, 12], F32, name="lnstat", tag="lnstat")
            nc.vector.bn_stats(st[:, 0:6], src[:, 0:512])
            nc.vector.bn_stats(st[:, 6:12], src[:, 512:1024])
            ag = stat.tile([128, 2], F32, name="lnag", tag="lnag")
            nc.vector.bn_aggr(ag[:], st[:])
            sd = stat.tile([128, 1], F32, name="lnsd", tag="lnsd")
            nc.scalar.activation(sd[:], ag[:, 1:2], AF.Sqrt, bias=eps_t[:])
            rstd = stat.tile([128, 1], F32, name="lnrstd", tag="lnrstd")
            nc.vector.reciprocal(rstd[:], sd[:])
            pool = out_pool or (full if out_dt == F32 else halfp)
            o = pool.tile([128, H], out_dt, name=out_tag,
                          tag="big" if out_dt == F32 else "bighalf")
            apply_engine.tensor_scalar(o[:], src[:], ag[:, 0:1],
                                       rstd[:], op0=OP.subtract, op1=OP.mult)
            return o

        def layernorm(src_tiles, out_dt, out_tag, apply_engine):
            return [layernorm_tile(src_tiles[mt], out_dt, out_tag, apply_engine)
                    for mt in range(TC)]

        def transpose_act(src_tiles, out_tag, fp32=False, engines=("v", "s")):
            """natural [T,H] tiles -> 4 e4m3 pair tiles [128, 2T]
            (pair j: cols [0:T]=H-chunk 2j, [T:2T]=chunk 2j+1).
            bf16 sources pack 4 transposed chunks per [128,1024]-f32 psum via
            a bf16 bitcast view; f32 sources use 2 chunks per psum."""
            out_tiles = []
            if not fp32:
                for hg in range(2):
                    pt = pspair.tile([128, 1024], F32, name="spair", tag="sp")
                    ptb = pt[:].bitcast(BF16)   # [128, 2048] bf16
                    for mt in range(TC):
                        for k in range(4):
                            hh = hg * 4 + k
                            nc.tensor.transpose(
                                ptb[:, k * 512 + mt * 128: k * 512 + (mt + 1) * 128],
                                src_tiles[mt][:, hh * 128:(hh + 1) * 128],
                                identb[:])
                    for j2 in range(2):
                        o = e4p.tile([128, 2 * T], E4, name=out_tag, tag="e4pair")
                        if engines[j2] == "v":
                            nc.vector.tensor_copy(o[:], ptb[:, j2 * 1024:(j2 + 1) * 1024])
                        else:
                            nc.scalar.copy(o[:], ptb[:, j2 * 1024:(j2 + 1) * 1024])
                        out_tiles.append(o)
            else:
                for hg in range(4):
                    pt = pspair.tile([128, 1024], F32, name="spair", tag="sp")
                    for mt in range(TC):
                        for k in range(2):
                            hh = hg * 2 + k
                            nc.tensor.transpose(
                                pt[:, k * 512 + mt * 128: k * 512 + (mt + 1) * 128],
                                src_tiles[mt][:, hh * 128:(hh + 1) * 128],
                                ident[:])
                    o = e4p.tile([128, 2 * T], E4, name=out_tag, tag="e4pair")
                    if engines[hg % 2] == "v":
                        nc.vector.tensor_copy(o[:], pt[:])
                    else:
                        nc.scalar.copy(o[:], pt[:])
                    out_tiles.append(o)
            return [o[:].rearrange("p (two f) -> p two f", two=2)
                    for o in out_tiles], out_tiles

        def proj_T_pair(w_tiles, actT, out_tag, evict, mo_pairs=None):
            """(act @ W)^T as raw pair tiles [128, 2T] (cols [0:T]=chunk 2mp).
            evict(pspair, out_tile, mp) writes the FD-1024 eviction."""
            out_tiles = []
            for mp in (mo_pairs if mo_pairs is not None else range(HP)):
                p = pspair.tile([128, 1024], F32, name="spair", tag="sp")
                for half in range(2):
                    mo = 2 * mp + half
                    for j in range(4):
                        nc.tensor.matmul(
                            p[:, half * 512:(half + 1) * 512],
                            w_tiles[j][:, :, mo * 128:(mo + 1) * 128],
                            actT[j][:],
                            start=(j == 0), stop=(j == 3), perf_mode=DR)
                o = e4p.tile([128, 2 * T], E4, name=out_tag, tag="e4pair")
                evict(p, o, mp)
                out_tiles.append(o)
            return out_tiles

        def proj_nat_pair(w_tiles, actT, n_list=(0, 1)):
            """natural-layout projection: yields (mt, pspair [128, 1024])."""
            for mt in range(TC):
                p = pspair.tile([128, 1024], F32, name="spair", tag="sp")
                for n in n_list:
                    for j in range(4):
                        nc.tensor.matmul(
                            p[:, n * 512:(n + 1) * 512],
                            actT[j][:, :, mt * 128:(mt + 1) * 128],
                            w_tiles[j][:, :, n * 512:(n + 1) * 512],
                            start=(j == 0), stop=(j == 3), perf_mode=DR)
                yield mt, p

        def make_vaug_tiles():
            vaug_tiles = []
            for mt in range(TC):
                vt = vaugp.tile([128, VA_ROW], E4, name="vt", tag="vaug")
                nc.vector.memset(vt[:], ONE_C)
                vaug_tiles.append(vt)
            return vaug_tiles

        def vaug_dst(vt, halfk=None):
            src = vt[:] if halfk is None else \
                vt[:, halfk * (VA_ROW // 2):(halfk + 1) * (VA_ROW // 2)]
            return src.rearrange("p (hd c) -> p hd c", c=DH + 1)[:, :, 0:DH]

        def proj_vaug(w_tiles, actT, vaug_tiles):
            for mt, p in proj_nat_pair(w_tiles, actT):
                dst = vaug_dst(vaug_tiles[mt])
                if mt % 2 == 0:
                    nc.scalar.activation(dst, p[:], AF.Copy, scale=1.0 / WS)
                else:
                    nc.vector.tensor_scalar(dst, p[:], 1.0 / WS, None, op0=OP.mult)

        def emit_kv(kT_tiles, vaug_tiles, ag_in, halfk=None):
            """kT pair tiles + vaug tiles -> ag DRAM buffer (bf16-typed)."""
            hps = range(HC) if halfk is None else range(4 * halfk, 4 * halfk + 4)
            for i, hp in enumerate(hps):
                mp, half = hp // 2, hp % 2
                src = kT_tiles[mp][:, half * T:(half + 1) * T].bitcast(BF16)
                nc.sync.dma_start(
                    ag_in[i * KT_CH:(i + 1) * KT_CH]
                    .rearrange("(p f) -> p f", p=128), src)
            ktl = len(list(hps)) * KT_CH
            for mt in range(TC):
                src = vaug_tiles[mt][:] if halfk is None else \
                    vaug_tiles[mt][:, halfk * (VA_ROW // 2):
                                   (halfk + 1) * (VA_ROW // 2)]
                w = src.free_size() // 2
                nc.sync.dma_start(
                    ag_in[ktl + mt * (128 * w):ktl + (mt + 1) * (128 * w)]
                    .rearrange("(p f) -> p f", p=128), src.bitcast(BF16))

        # =====================================================================
        # attention inner loop
        # =====================================================================
        def prepare_attention(kt_src, vt_src):
            pre = {"kts": {}, "vts": {}}

            def load_kt(hp):
                kt = ktp.tile([128, 2 * T], E4, name="kt", tag="kt")
                for sl in range(2):
                    nc.sync.dma_start(
                        kt[:, sl * T:(sl + 1) * T].bitcast(BF16),
                        kt_src(hp, sl))
                pre["kts"][hp] = kt

            def load_vts(hpp):
                lst = []
                for tkp in range(HC // 2):
                    vt = vtp.tile([128, 2 * VA_P], E4, name="vt", tag="vt")
                    for s in range(2):
                        tk = 2 * tkp + s
                        sl, ro = tk // TC, (tk % TC) * 128
                        nc.sync.dma_start(
                            vt[:, s * VA_P:s * VA_P + VA_G].bitcast(BF16),
                            vt_src(hpp, sl, ro))
                    lst.append(vt[:].rearrange("p (two f) -> p two f", two=2))
                pre["vts"][hpp] = lst

            pre["load_kt"] = load_kt
            pre["load_vts"] = load_vts
            load_kt(0)
            load_kt(1)
            load_vts(0)
            return pre

        def emit_attention(qT_pairs, pre, wo_tiles, resid_tiles,
                           out_tag, per_mt_hook=None):
            """64 flat iterations (8 hp x 8 tk); PV every other iteration via
            fp8 DoubleRow over key-tile pairs."""
            NIT = HC * HC  # 64
            oT_pairs = [e4p.tile([128, 2 * T], E4, name="oT", tag="e4pair")
                        for _ in range(HP)]
            kts = pre["kts"]
            vts = pre["vts"]
            load_kt = pre["load_kt"]
            load_vts = pre["load_vts"]
            accs = {}
            pairs = [None] * NIT
            pps = {}
            tails = {}

            def stage_scores(it):
                hp, tk = it // HC, it % HC
                if tk == 0:
                    if hp + 2 < HC:
                        load_kt(hp + 2)
                    accs[hp] = (ps.tile([128, 512], F32, name="oacc", tag="ps"),
                                ps.tile([128, 512], F32, name="oacc", tag="ps"))
                if tk == 4 and hp % 2 == 1 and hp // 2 + 1 < 4:
                    load_vts(hp // 2 + 1)
                pair = pspair.tile([128, 1024], F32, name="spair", tag="sp")
                kt = kts[hp]
                mp, half = hp // 2, hp % 2
                qT = qT_pairs[mp]
                nc.tensor.matmul(pair[:, 0:512],
                                 kt[0:64, tk * 128:(tk + 1) * 128],
                                 qT[0:64, half * T:(half + 1) * T],
                                 start=True, stop=True, tile_position=(0, 0))
                nc.tensor.matmul(pair[:, 512:1024],
                                 kt[64:128, tk * 128:(tk + 1) * 128],
                                 qT[64:128, half * T:(half + 1) * T],
                                 start=True, stop=True, tile_position=(64, 0))
                pairs[it] = pair

            ESP = 640   # scalar handles [0:ESP], DVE [ESP:1024] of each tile

            def stage_exp(it):
                tk = it % HC
                if tk % 2 == 0:
                    pp = ppp.tile([128, 2 * 1024], E5, name="pp", tag="pp")
                    pps[it // 2] = pp
                else:
                    pp = pps[it // 2]
                base = (tk % 2) * 1024
                nc.scalar.activation(pp[:, base:base + ESP],
                                     pairs[it][:, 0:ESP], AF.Exp,
                                     scale=1.0 / A5)
                nc.vector.tensor_scalar(
                    pp[:, base + ESP:base + 1024].bitcast(I8),
                    pairs[it][:, ESP:1024], B5, None, op0=OP.add)
                pairs[it] = None

            def stage_pv(it, step):
                hp, tk = it // HC, it % HC
                if tk % 2 == 0:
                    return
                hpp, i = hp // 2, hp % 2
                tkp = tk // 2
                vt = vts[hpp][tkp]
                pp3 = pps[it // 2][:].rearrange("p (two f) -> p two f", two=2)
                oa, ob = accs[hp]
                for hi in range(2):
                    head = 2 * i + hi
                    dst = oa if hi == 0 else ob
                    nc.tensor.matmul(
                        dst[0:DH + 1, :],
                        vt[:, :, head * (DH + 1):(head + 1) * (DH + 1)],
                        pp3[:, :, hi * 512:(hi + 1) * 512],
                        start=(tkp == 0), stop=(tkp == 3), perf_mode=DR)
                pps[it // 2] = None
                if tk == HC - 1:
                    schedule_tail(hp, step)

            def schedule_tail(hp, step):
                oa, ob = accs.pop(hp)
                mp, half = hp // 2, hp % 2
                st = {}

                def t0():
                    st["den"] = rowp.tile([1, 2 * T], F32, name="den", tag="den")
                    nc.scalar.copy(st["den"][:, 0:T], oa[DH:DH + 1, :])
                    nc.vector.tensor_copy(st["den"][:, T:2 * T], ob[DH:DH + 1, :])

                def t1():
                    st["recip"] = rowp.tile([1, 2 * T], F32, name="recip", tag="recip")
                    nc.vector.reciprocal_approx_fast(st["recip"][:], st["den"][:])

                def t2():
                    st["recr"] = rowp.tile([1, 2 * T], BF16, name="recr", tag="recr")
                    nc.scalar.copy(st["recr"][:], st["recip"][:])

                def t3():
                    st["bcs"] = bcsp.tile([64, 2 * T], BF16, name="bcs", tag="bcs")
                    nc.gpsimd.partition_broadcast(st["bcs"][:], st["recr"][:])

                def t4():
                    oT = oT_pairs[mp]
                    nc.vector.tensor_mul(oT[0:64, half * T:(half + 1) * T],
                                         oa[0:64, :], st["bcs"][:, 0:T])
                    nc.vector.tensor_mul(oT[64:128, half * T:(half + 1) * T],
                                         ob[0:64, :], st["bcs"][:, T:2 * T])

                for off, fn in ((1, t0), (2, t1), (3, t2), (4, t3), (6, t4)):
                    tails.setdefault(step + off, []).append(fn)

            for step in range(NIT + 8):
                if step < NIT:
                    stage_scores(step)
                if 1 <= step <= NIT:
                    stage_exp(step - 1)
                if 2 <= step <= NIT + 1:
                    stage_pv(step - 2, step)
                for fn in tails.pop(step, ()):
                    fn()

            # oT pair cols [0:T] = hp even chunk = H rows [256mp:256mp+128]
            oT3 = [o[:].rearrange("p (two f) -> p two f", two=2)
                   for o in oT_pairs]
            out_tiles = [full.tile([128, H], F32, name=out_tag, tag="big")
                         for _ in range(TC)]
            for mt, p in proj_nat_pair(wo_tiles, oT3):
                nc.vector.scalar_tensor_tensor(
                    out_tiles[mt][:], p[:], 1.0 / (16.0 * WS),
                    resid_tiles[mt][:], op0=OP.mult, op1=OP.add)
                if per_mt_hook is not None:
                    per_mt_hook(mt, out_tiles[mt])
            return out_tiles

        # =====================================================================
        # Phase 1: silu -> siluT -> ada MMs (PE) while hn/xln LN chains (DVE)
        # =====================================================================
        with nc.named_scope("p1_ada"):
            silu = []
            for mt in range(TC):
                sl_t = halfp.tile([128, H], BF16, name="silu", tag="bighalf")
                nc.vector.tensor_add(sl_t[:], t_sb[mt][:], h_sb[mt][:])
                nc.scalar.activation(sl_t[:], sl_t[:], AF.Silu)
                silu.append(sl_t)
            siluT, _ = transpose_act(silu, "siluT")

            shift_sb = [bmod.tile([128, H], BF16, name="shift", tag="mod") for _ in range(TC)]
            scale1_sb = [bmod.tile([128, H], BF16, name="scale1", tag="mod") for _ in range(TC)]
            for mt, p in proj_nat_pair(ada_tiles0, siluT):
                nc.scalar.activation(shift_sb[mt][:], p[:], AF.Copy,
                                     scale=1.0 / WS, bias=0.0)
            ada_tiles1 = load_weight("ada_w", half=1)
            for mt, p in proj_nat_pair(ada_tiles1, siluT):
                nc.scalar.activation(scale1_sb[mt][:], p[:], AF.Copy,
                                     scale=1.0 / WS, bias=1.0)

        # =====================================================================
        # Phase 1b: cross K/V matmuls (fills PE while xln/mods run on DVE);
        # the AllGather itself is issued later, after the self-AG halves.
        # =====================================================================
        def kv_evict(p, o, mp):
            if mp % 2 == 0:
                nc.scalar.activation(o[:], p[:], AF.Copy, scale=1.0 / WS)
            else:
                nc.vector.tensor_scalar(o[:], p[:], 1.0 / WS, None, op0=OP.mult)

        # =====================================================================
        # Phase 2b: hidden_in = xln * (1+scale) + shift; hinT
        # =====================================================================
        with nc.named_scope("p2b_hin"):
            xln = layernorm(x_sb, BF16, "xln", nc.vector)
            hin = []
            for mt in range(TC):
                nc.vector.tensor_mul(xln[mt][:], xln[mt][:], scale1_sb[mt][:])
                nc.vector.tensor_add(xln[mt][:], xln[mt][:], shift_sb[mt][:])
                hin.append(xln[mt])
            hinT, _ = transpose_act(hin, "hinT", engines=("s", "s"))


        # ag source maps -----------------------------------------------------
        def kt_src_s(hp, sl):
            half, hpl = hp // 4, hp % 4
            return ag_so[half][sl * HALF_LEN + hpl * KT_CH:
                               sl * HALF_LEN + (hpl + 1) * KT_CH] \
                .rearrange("(p f) -> p f", p=128)

        def vt_src_s(hpp, sl, ro):
            half, hl = hpp // 2, hpp % 2
            v = ag_so[half][sl * HALF_LEN + HALF_KT:
                            sl * HALF_LEN + HALF_KT + HALF_VA] \
                .rearrange("(tt f) -> tt f", tt=T)
            return v[ro:ro + 128, hl * (VA_G // 2):(hl + 1) * (VA_G // 2)]

        def kt_src_c(hp, sl):
            return ag_out_c[sl * RANK_LEN + hp * KT_CH:
                            sl * RANK_LEN + (hp + 1) * KT_CH] \
                .rearrange("(p f) -> p f", p=128)

        def vt_src_c(hpp, sl, ro):
            v = ag_out_c[sl * RANK_LEN + KT_LEN:
                         sl * RANK_LEN + KT_LEN + VA_LEN] \
                .rearrange("(tt f) -> tt f", tt=T)
            return v[ro:ro + 128, hpp * (VA_G // 2):(hpp + 1) * (VA_G // 2)]

        wsk = load_weight("Wsk")
        wsv = load_weight("Wsv")
        # =====================================================================
        # Phase 3: self K/V halves + AllGather(self)
        # =====================================================================
        with nc.named_scope("p3_selfkv"):
            vaug_s = make_vaug_tiles()
            ksT = [None] * HP
            ag_s = (ag_in_s0, ag_in_s1)
            ag_so = (ag_out_s0, ag_out_s1)
            for halfk in range(2):
                part = proj_T_pair(wsk, hinT, "ksT", kv_evict,
                                   mo_pairs=[2 * halfk, 2 * halfk + 1])
                ksT[2 * halfk] = part[0]
                ksT[2 * halfk + 1] = part[1]
                for mt, p in proj_nat_pair(wsv, hinT, n_list=(halfk,)):
                    dst = vaug_dst(vaug_s[mt], halfk=halfk)
                    src = p[:, halfk * 512:(halfk + 1) * 512]
                    if mt % 2 == 0:
                        nc.scalar.activation(dst, src, AF.Copy, scale=1.0 / WS)
                    else:
                        nc.vector.tensor_scalar(dst, src, 1.0 / WS, None, op0=OP.mult)
                agi = ag_s[halfk]
                emit_kv(ksT, vaug_s, agi, halfk=halfk)
                nc.gpsimd.collective_compute(
                    "AllGather", OP.bypass,
                    replica_groups=[[0, 1], [2, 3], [4, 5], [6, 7]],
                    ins=[agi.opt()], outs=[ag_so[halfk].opt()])
                if halfk == 0:
                    pre_s = prepare_attention(kt_src_s, vt_src_s)

        wsq = load_weight("Wsq")
        wso = load_weight("Wso")

        def q_evict(bc):
            def ev(p, o, mp):
                nc.vector.tensor_mul(o[:], p[:], bc[:])
            return ev

        with nc.named_scope("p4_qself"):
            em_bc = mask_bcast("em", "em", 0.125 * A5 / WS)
            qsT = proj_T_pair(wsq, hinT, "qsT", q_evict(em_bc))

        wck = load_weight("Wck")
        wcv = load_weight("Wcv")
        with nc.named_scope("p1b_crosskv"):
            hn = layernorm(h_sb, BF16, "hn", nc.vector)
            hnT, _ = transpose_act(hn, "hnT")
            kcT = proj_T_pair(wck, hnT, "kcT", kv_evict)
            vaug_c = make_vaug_tiles()
            proj_vaug(wcv, hnT, vaug_c)



        # =====================================================================
        # Phase 4: self attention; cross q
        # =====================================================================
        wcq = load_weight("Wcq")
        wco = load_weight("Wco")
        with nc.named_scope("p5_selfattn"):
            ln2o = [None] * TC

            def ln2_hook(mt, tile_):
                ln2o[mt] = layernorm_tile(tile_, BF16, "ln2o", nc.vector)

            hidden_in = emit_attention(qsT, pre_s, wso,
                                       x_sb, "res1", per_mt_hook=ln2_hook)

        with nc.named_scope("p2_crossag"):
            emit_kv(kcT, vaug_c, ag_in_c)
            nc.gpsimd.collective_compute(
                "AllGather", OP.bypass,
                replica_groups=[[0, 1], [2, 3], [4, 5], [6, 7]],
                ins=[ag_in_c.opt()], outs=[ag_out_c.opt()])
        w1 = load_weight("ffn_w1")
        pre_c = prepare_attention(kt_src_c, vt_src_c)
        with nc.named_scope("p6_ln2q"):
            mk_bc = mask_bcast("mk", "mk", 0.125 * A5 / WS)
            ln2T, _ = transpose_act(ln2o, "ln2T", engines=("s", "s"))
            qcT = proj_T_pair(wcq, ln2T, "qcT", q_evict(mk_bc))

        # =====================================================================
        # Phase 5: cross attention -> hidden_mid
        # =====================================================================
        w2 = load_weight("ffn_w2")
        with nc.named_scope("p7_crossattn"):
            ln3o = [None] * TC

            def ln3_hook(mt, tile_):
                ln3o[mt] = layernorm_tile(tile_, F32, "ln3o", nc.vector,
                                          out_pool=full)

            hidden_mid = emit_attention(qcT, pre_c, wco,
                                        hidden_in, "hmid", per_mt_hook=ln3_hook)

        # =====================================================================
        # Phase 6: FFN + final LN + output (mt-pipelined tail)
        # =====================================================================
        with nc.named_scope("p8_ffn"):
            hoT, _ = transpose_act(ln3o, "hoT", fp32=True, engines=("s", "s"))
            midT_raw = []
            for mp in range(HP):
                p = pspair.tile([128, 1024], F32, name="spair", tag="sp")
                for half in range(2):
                    mo = 2 * mp + half
                    for j in range(4):
                        nc.tensor.matmul(
                            p[:, half * 512:(half + 1) * 512],
                            w1[j][:, :, mo * 128:(mo + 1) * 128],
                            hoT[j][:],
                            start=(j == 0), stop=(j == 3), perf_mode=DR)
                o = e4p.tile([128, 2 * T], E4, name="midT", tag="e4pair")
                if mp % 2 == 0:
                    nc.scalar.activation(o[:], p[:], AF.Relu, scale=1.0 / WS)
                else:
                    nc.vector.tensor_scalar(o[:], p[:], 1.0 / WS, 0.0,
                                            op0=OP.mult, op1=OP.max)
                midT_raw.append(o)
            midT = [o[:].rearrange("p (two f) -> p two f", two=2)
                    for o in midT_raw]

            for mt, p in proj_nat_pair(w2, midT):
                ffres = full.tile([128, H], F32, name="ffres", tag="big")
                nc.vector.scalar_tensor_tensor(
                    ffres[:], p[:], 1.0 / WS,
                    ln3o[mt][:], op0=OP.mult, op1=OP.add)
                lnfo = layernorm_tile(ffres, F32, "lnfo", nc.vector,
                                      out_pool=full)
                nc.vector.tensor_add(lnfo[:], lnfo[:], hidden_mid[mt][:])
                nc.sync.dma_start(out_ext.ap()[mt * 128:(mt + 1) * 128, :],
                                  lnfo[:])


def _bf16(a):
    import ml_dtypes
    return np.asarray(a, np.float32).astype(ml_dtypes.bfloat16)


def _e4(a):
    import ml_dtypes
    return np.clip(np.asarray(a, np.float32), -240, 240).astype(
        ml_dtypes.float8_e4m3)


def _pack_w(w):
    """[K, N] -> [512, 2N] e4m3 pair-packed, prescaled x32."""
    w = np.asarray(w, np.float32) * WS
    K, N = w.shape
    wp = w.reshape(4, 2, 128, N).transpose(0, 2, 1, 3).reshape(512, 2 * N)
    return np.ascontiguousarray(_e4(wp))


def kernel(**inputs):
    global LAST_RESULT
    if "nc" not in _BUILD_CACHE:
        _BUILD_CACHE["nc"] = _build()
    nc = _BUILD_CACHE["nc"]

    x = np.asarray(inputs["x"], np.float32)
    h = np.asarray(inputs["h"], np.float32)
    t = np.asarray(inputs["t"], np.float32)
    em = np.asarray(inputs["extent_mask"], np.int32)
    mk = np.asarray(inputs["mask"], np.int32)

    common = {}
    for nm in W_NAMES + ["ada_w"]:
        common[nm] = _pack_w(inputs[nm])
    common["c_ones"] = _bf16(np.ones((1, T)))
    common["c_ident"] = np.eye(128, dtype=np.float32)
    common["c_identb"] = _bf16(np.eye(128))

    in_maps = []
    for c in range(NC):
        b, half = c // 2, c % 2
        s0 = half * T
        m = dict(common)
        m["x"] = np.ascontiguousarray(_bf16(x[b, s0:s0 + T]))
        m["h"] = np.ascontiguousarray(_bf16(h[b, s0:s0 + T]))
        m["t"] = np.ascontiguousarray(_bf16(t[b, s0:s0 + T]))
        m["em"] = np.ascontiguousarray(em[b, s0:s0 + T].reshape(1, T))
        m["mk"] = np.ascontiguousarray(mk[b, s0:s0 + T].reshape(1, T))
        in_maps.append(m)

    trace = bool(os.environ.get("BASS_TRACE_KERNEL"))
    if trace:
        _install_ntff_hook()
    try:
        res = bass_utils.run_bass_kernel_spmd(
            nc, in_maps, core_ids=list(range(NC)), trace=trace)
    except Exception:
        import time
        time.sleep(20)
        res = bass_utils.run_bass_kernel_spmd(
            nc, in_maps, core_ids=list(range(NC)), trace=trace)
    LAST_RESULT = res

    out = np.empty((B, S, H), np.float32)
    for c in range(NC):
        b, half = c // 2, c % 2
        out[b, half * T:(half + 1) * T] = res.results[c]["out"]
    return out


def _install_ntff_hook():
    import sys, types
    if 'antenv.axon_hooks' in sys.modules:
        return
    mod = types.ModuleType("antenv.axon_hooks")
    mod._hook = None
    def set_axon_ntff_profile_hook(h): mod._hook = h
    def get_axon_ntff_profile_hook(): return mod._hook
    mod.set_axon_ntff_profile_hook = set_axon_ntff_profile_hook
    mod.get_axon_ntff_profile_hook = get_axon_ntff_profile_hook
    sys.modules['antenv.axon_hooks'] = mod
    import antenv
    antenv.axon_hooks = mod
    try:
        from trn_agent_boot.trn_boot import _ntff_profile_via_ctypes
        mod.set_axon_ntff_profile_hook(
            _ntff_profile_via_ctypes('/opt/axon/libaxon_pjrt.so'))
    except Exception:
        pass


# revision 19
# speedup vs baseline: 1.1568x; 1.1568x over previous
"""AdaLN transformer block on 8 TRN2 NeuronCores (Bass/Tile), fp8 edition.

Sharding: 4096 tokens (B*S) split 8 ways -> 512 tokens/core; cores (2b, 2b+1)
own batch element b. Attention needs full-sequence K/V per batch element:
pairwise AllGather (replica groups [[0,1],[2,3],[4,5],[6,7]]) of fp8 K^T /
V(+1/16 col).

Precision: all projections run in fp8 e4m3 with DoubleRow perf mode (two
128-row contraction chunks per matmul -> 2x PE throughput; weights are
host-prescaled x32 and pair-packed [128,2,N]). Scores use e4m3 qT/kT with the
softmax exp prescale A5=4/ln2 folded into the q eviction; probabilities are
e5m2 via ScalarE exact exp (even key tiles) and a DVE int8 Schraudolph trick
(odd key tiles, round-to-nearest, tuned B5). PV runs fp8 DoubleRow over
key-tile pairs; the V ones-column is 1/16 so the reciprocal fold lands oT in
e4m3 range (x16). The f32 spine (x residual, LN stats, PSUM accumulation,
softmax denominators) keeps total rel err ~1e-2 (tolerance 2e-2).
"""
import os
import math
import numpy as np

import concourse.bass as bass
import concourse.bacc as bacc
import concourse.tile as tile
import concourse.mybir as mybir
from concourse import bass_utils

B, S, H, NH = 4, 1024, 1024, 16
DH = H // NH               # 64
EPS = 1e-5
NC = 8
T = (B * S) // NC          # 512 tokens per core
TC = T // 128              # 4
HC = H // 128              # 8
HP = HC // 2               # 4 pair tiles per H-contraction

VA_G = 4 * (DH + 1)        # 260: one 4-head group's v_aug row chunk (fp8)
VA_ROW = 4 * VA_G          # 1040: full v_aug row (16 heads)
VA_P = 272                 # padded pair-slot stride in vt tiles (mult of 16)
KT_CH = 128 * T // 2       # 32768: one [128,T] fp8 kT chunk in bf16 elems
KT_LEN = HC * KT_CH        # 262144
VA_LEN = T * VA_ROW // 2   # 266240
RANK_LEN = KT_LEN + VA_LEN
HALF_KT = 4 * KT_CH
HALF_VA = T * (VA_ROW // 2) // 2
HALF_LEN = HALF_KT + HALF_VA

F32 = mybir.dt.float32
BF16 = mybir.dt.bfloat16
E4 = mybir.dt.float8e4
E5 = mybir.dt.float8e5
I8 = mybir.dt.int8
I32 = mybir.dt.int32
AF = mybir.ActivationFunctionType
OP = mybir.AluOpType
DR = mybir.MatmulPerfMode.DoubleRow

WS = 32.0                  # host weight prescale (power of 2)
A5 = 4.0 / math.log(2.0)   # e5m2 Schraudolph scale (folded into q eviction)
B5 = 59.76                 # tuned for round-to-nearest f32->int8
ONE_C = 1.0 / 16.0         # v_aug ones column; recip fold scales oT x16

W_NAMES = ["Wsq", "Wsk", "Wsv", "Wso", "Wcq", "Wck", "Wcv", "Wco",
           "ffn_w1", "ffn_w2"]

LAST_RESULT = None
_BUILD_CACHE = {}


def _build():
    nc = bacc.Bacc("TRN2", target_bir_lowering=False, debug=False,
                   num_devices=NC)
    ext = {}
    for nm in ["x", "h", "t"]:
        ext[nm] = nc.dram_tensor(nm, [T, H], BF16, kind="ExternalInput")
    ext["em"] = nc.dram_tensor("em", [1, T], I32, kind="ExternalInput")
    ext["mk"] = nc.dram_tensor("mk", [1, T], I32, kind="ExternalInput")
    for nm in W_NAMES:
        ext[nm] = nc.dram_tensor(nm, [4 * 128, 2 * H], E4, kind="ExternalInput")
    ext["ada_w"] = nc.dram_tensor("ada_w", [4 * 128, 2 * 2 * H], E4,
                                  kind="ExternalInput")
    ext["c_ones"] = nc.dram_tensor("c_ones", [1, T], BF16, kind="ExternalInput")
    ext["c_ident"] = nc.dram_tensor("c_ident", [128, 128], F32, kind="ExternalInput")
    ext["c_identb"] = nc.dram_tensor("c_identb", [128, 128], BF16, kind="ExternalInput")
    out_ext = nc.dram_tensor("out", [T, H], F32, kind="ExternalOutput")

    with tile.TileContext(nc) as tc:
        _emit(nc, tc, ext, out_ext)
    nc.compile()
    return nc


def _emit(nc, tc, ext, out_ext):
    import contextlib
    ctx = contextlib.ExitStack()
    with ctx:
        full = ctx.enter_context(tc.tile_pool(name="full", bufs=13))
        halfp = ctx.enter_context(tc.tile_pool(name="halfp", bufs=16))
        e4p = ctx.enter_context(tc.tile_pool(name="e4p", bufs=14))
        wpool = ctx.enter_context(tc.tile_pool(name="wpool", bufs=16))
        bmod = ctx.enter_context(tc.tile_pool(name="bmod", bufs=8))
        ktp = ctx.enter_context(tc.tile_pool(name="ktp", bufs=3))
        vtp = ctx.enter_context(tc.tile_pool(name="vtp", bufs=8))
        ppp = ctx.enter_context(tc.tile_pool(name="ppp", bufs=3))
        bcsp = ctx.enter_context(tc.tile_pool(name="bcsp", bufs=2))
        vaugp = ctx.enter_context(tc.tile_pool(name="vaugp", bufs=4))
        smalls = ctx.enter_context(tc.tile_pool(name="smalls", bufs=1))
        stat = ctx.enter_context(tc.tile_pool(name="stat", bufs=8))
        rowp = ctx.enter_context(tc.tile_pool(name="rowp", bufs=2))
        ps = ctx.enter_context(tc.tile_pool(name="ps", bufs=4, space="PSUM"))
        pspair = ctx.enter_context(tc.tile_pool(name="pspair", bufs=2, space="PSUM"))
        dram = ctx.enter_context(tc.tile_pool(name="dram", bufs=1, space="DRAM"))

        # ---------------- constants ----------------
        ones = smalls.tile([1, T], BF16, name="ones", tag="ones")
        nc.sync.dma_start(ones[:], ext["c_ones"].ap())
        ident = smalls.tile([128, 128], F32, name="ident", tag="ident")
        nc.sync.dma_start(ident[:], ext["c_ident"].ap())
        identb = smalls.tile([128, 128], BF16, name="identb", tag="identb")
        nc.sync.dma_start(identb[:], ext["c_identb"].ap())
        eps_t = smalls.tile([128, 1], F32, name="eps_t", tag="eps_t")
        nc.vector.memset(eps_t[:], EPS)
        # HAM warmup: keep PE busy during the initial input DMA.
        wu = ps.tile([128, 512], F32, name="wu", tag="ps")
        wu_src = ident[:].bitcast(BF16)
        for _ in range(30):
            nc.tensor.matmul(wu[:, 0:256], wu_src[:, 0:128], wu_src[:],
                             start=True, stop=True)

        # skew absorber: tiny AllGather at t=0; DVE syncs on it so all cores
        # align before the heavy phases (later collectives then see ~0 skew).
        dummy_in = dram.tile([128], BF16, name="dummy_in", tag="dummy_in")
        dummy_out = dram.tile([256], BF16, name="dummy_out", tag="dummy_out")
        dsb = smalls.tile([1, 128], BF16, name="dsb", tag="dsb")
        nc.vector.memset(dsb[:], 0.0)
        nc.sync.dma_start(dummy_in.rearrange("(p f) -> p f", p=1), dsb[:])
        nc.gpsimd.collective_compute(
            "AllGather", OP.bypass,
            replica_groups=[[0, 1], [2, 3], [4, 5], [6, 7]],
            ins=[dummy_in.opt()], outs=[dummy_out.opt()])
        dsb2 = smalls.tile([1, 256], BF16, name="dsb2", tag="dsb2")
        nc.sync.dma_start(dsb2[:], dummy_out.rearrange("(p f) -> p f", p=1))
        dsb3 = smalls.tile([1, 256], BF16, name="dsb3", tag="dsb3")
        nc.vector.tensor_copy(dsb3[:], dsb2[:])

        # ---------------- input loads (h first: needed earliest) ----------
        h_sb, t_sb, x_sb = [], [], []
        for mt in range(TC):
            th = halfp.tile([128, H], BF16, name="h", tag="bighalf")
            nc.sync.dma_start(th[:], ext["h"].ap()[mt * 128:(mt + 1) * 128, :])
            h_sb.append(th)

        def load_weight(nm, half=None):
            """fp8 pair-packed weight: 4 tiles [128, 2, N] (rearranged APs)."""
            n = 2 * H if nm == "ada_w" else H
            if half is not None:
                n = n // 2
            tiles = []
            for j in range(4):
                t_ = wpool.tile([128, 2 * n], E4, name="w_" + nm, tag="w")
                if half is not None:
                    t3 = t_[:].rearrange("p (two f) -> p two f", two=2)
                    for s in range(2):
                        nc.sync.dma_start(
                            t3[:, s, :],
                            ext[nm].ap()[j * 128:(j + 1) * 128,
                                         s * 2 * H + half * n:
                                         s * 2 * H + (half + 1) * n])
                else:
                    nc.sync.dma_start(t_[:], ext[nm].ap()[j * 128:(j + 1) * 128, :])
                tiles.append(t_[:].rearrange("p (two f) -> p two f", two=2))
            return tiles

        for mt in range(TC):
            tt = halfp.tile([128, H], BF16, name="tin", tag="bighalf")
            nc.sync.dma_start(tt[:], ext["t"].ap()[mt * 128:(mt + 1) * 128, :])
            t_sb.append(tt)
        for mt in range(TC):
            tx = halfp.tile([128, H], BF16, name="x", tag="bighalf")
            nc.sync.dma_start(tx[:], ext["x"].ap()[mt * 128:(mt + 1) * 128, :])
            x_sb.append(tx)
        ada_tiles0 = load_weight("ada_w", half=0)

        def mask_bcast(name, tagn, scale):
            """[128, 2T] bf16 broadcast of mask*scale (same mask both halves)."""
            mi = smalls.tile([1, T], I32, name=tagn + "_i", tag=tagn + "_i")
            nc.sync.dma_start(mi[:], ext[name].ap())
            mf = smalls.tile([1, T], F32, name=tagn + "_f", tag=tagn + "_f")
            nc.vector.tensor_copy(mf[:], mi[:])
            mr = smalls.tile([1, T], BF16, name=tagn + "_r", tag=tagn + "_r")
            nc.vector.tensor_scalar_mul(mr[:], mf[:], scale)
            bc = smalls.tile([128, 2 * T], BF16, name=tagn + "_bc", tag=tagn + "_bc")
            p = pspair.tile([128, 1024], F32, name="spair", tag="sp")
            nc.tensor.matmul(p[:, 0:512], ones[:, 0:128], mr[:], start=True, stop=True)
            nc.tensor.matmul(p[:, 512:1024], ones[:, 0:128], mr[:], start=True, stop=True)
            nc.vector.tensor_copy(bc[:], p[:])
            return bc

        ag_in_s0 = dram.tile([HALF_LEN], BF16, name="agins0", tag="agins0")
        ag_out_s0 = dram.tile([2 * HALF_LEN], BF16, name="agouts0", tag="agouts0")
        ag_in_s1 = dram.tile([HALF_LEN], BF16, name="agins1", tag="agins1")
        ag_out_s1 = dram.tile([2 * HALF_LEN], BF16, name="agouts1", tag="agouts1")
        ag_in_c = dram.tile([RANK_LEN], BF16, name="aginc", tag="aginc")
        ag_out_c = dram.tile([2 * RANK_LEN], BF16, name="agoutc", tag="agoutc")

        # ---------------- helpers ----------------
        def layernorm_tile(src, out_dt, out_tag, apply_engine, out_pool=None):
            st = stat.tile([128, 12], F32, name="lnstat", tag="lnstat")
            nc.vector.bn_stats(st[:, 0:6], src[:, 0:512])
            nc.vector.bn_stats(st[:, 6:12], src[:, 512:1024])
            ag = stat.tile([128, 2], F32, name="lnag", tag="lnag")
            nc.vector.bn_aggr(ag[:], st[:])
            sd = stat.tile([128, 1], F32, name="lnsd", tag="lnsd")
            nc.scalar.activation(sd[:], ag[:, 1:2], AF.Sqrt, bias=eps_t[:])
            rstd = stat.tile([128, 1], F32, name="lnrstd", tag="lnrstd")
            nc.vector.reciprocal(rstd[:], sd[:])
            pool = out_pool or (full if out_dt == F32 else halfp)
            o = pool.tile([128, H], out_dt, name=out_tag,
                          tag="big" if out_dt == F32 else "bighalf")
            apply_engine.tensor_scalar(o[:], src[:], ag[:, 0:1],
                                       rstd[:], op0=OP.subtract, op1=OP.mult)
            return o

        def layernorm(src_tiles, out_dt, out_tag, apply_engine):
            return [layernorm_tile(src_tiles[mt], out_dt, out_tag, apply_engine)
                    for mt in range(TC)]

        def transpose_act(src_tiles, out_tag, fp32=False, engines=("v", "s")):
            """natural [T,H] tiles -> 4 e4m3 pair tiles [128, 2T]
            (pair j: cols [0:T]=H-chunk 2j, [T:2T]=chunk 2j+1).
            bf16 sources pack 4 transposed chunks per [128,1024]-f32 psum via
            a bf16 bitcast view; f32 sources use 2 chunks per psum."""
            out_tiles = []
            if not fp32:
                for hg in range(2):
                    pt = pspair.tile([128, 1024], F32, name="spair", tag="sp")
                    ptb = pt[:].bitcast(BF16)   # [128, 2048] bf16
                    for mt in range(TC):
                        for k in range(4):
                            hh = hg * 4 + k
                            nc.tensor.transpose(
                                ptb[:, k * 512 + mt * 128: k * 512 + (mt + 1) * 128],
                                src_tiles[mt][:, hh * 128:(hh + 1) * 128],
                                identb[:])
                    for j2 in range(2):
                        o = e4p.tile([128, 2 * T], E4, name=out_tag, tag="e4pair")
                        if engines[j2] == "v":
                            nc.vector.tensor_copy(o[:], ptb[:, j2 * 1024:(j2 + 1) * 1024])
                        else:
                            nc.scalar.copy(o[:], ptb[:, j2 * 1024:(j2 + 1) * 1024])
                        out_tiles.append(o)
            else:
                for hg in range(4):
                    pt = pspair.tile([128, 1024], F32, name="spair", tag="sp")
                    for mt in range(TC):
                        for k in range(2):
                            hh = hg * 2 + k
                            nc.tensor.transpose(
                                pt[:, k * 512 + mt * 128: k * 512 + (mt + 1) * 128],
                                src_tiles[mt][:, hh * 128:(hh + 1) * 128],
                                ident[:])
                    o = e4p.tile([128, 2 * T], E4, name=out_tag, tag="e4pair")
                    if engines[hg % 2] == "v":
                        nc.vector.tensor_copy(o[:], pt[:])
                    else:
                        nc.scalar.copy(o[:], pt[:])
                    out_tiles.append(o)
            return [o[:].rearrange("p (two f) -> p two f", two=2)
                    for o in out_tiles], out_tiles

        def proj_T_pair(w_tiles, actT, out_tag, evict, mo_pairs=None):
            """(act @ W)^T as raw pair tiles [128, 2T] (cols [0:T]=chunk 2mp).
            evict(pspair, out_tile, mp) writes the FD-1024 eviction."""
            out_tiles = []
            for mp in (mo_pairs if mo_pairs is not None else range(HP)):
                p = pspair.tile([128, 1024], F32, name="spair", tag="sp")
                for half in range(2):
                    mo = 2 * mp + half
                    for j in range(4):
                        nc.tensor.matmul(
                            p[:, half * 512:(half + 1) * 512],
                            w_tiles[j][:, :, mo * 128:(mo + 1) * 128],
                            actT[j][:],
                            start=(j == 0), stop=(j == 3), perf_mode=DR)
                o = e4p.tile([128, 2 * T], E4, name=out_tag, tag="e4pair")
                evict(p, o, mp)
                out_tiles.append(o)
            return out_tiles

        def proj_nat_pair(w_tiles, actT, n_list=(0, 1)):
            """natural-layout projection: yields (mt, pspair [128, 1024])."""
            for mt in range(TC):
                p = pspair.tile([128, 1024], F32, name="spair", tag="sp")
                for n in n_list:
                    for j in range(4):
                        nc.tensor.matmul(
                            p[:, n * 512:(n + 1) * 512],
                            actT[j][:, :, mt * 128:(mt + 1) * 128],
                            w_tiles[j][:, :, n * 512:(n + 1) * 512],
                            start=(j == 0), stop=(j == 3), perf_mode=DR)
                yield mt, p

        def make_vaug_tiles():
            vaug_tiles = []
            for mt in range(TC):
                vt = vaugp.tile([128, VA_ROW], E4, name="vt", tag="vaug")
                nc.vector.memset(vt[:], ONE_C)
                vaug_tiles.append(vt)
            return vaug_tiles

        def vaug_dst(vt, halfk=None):
            src = vt[:] if halfk is None else \
                vt[:, halfk * (VA_ROW // 2):(halfk + 1) * (VA_ROW // 2)]
            return src.rearrange("p (hd c) -> p hd c", c=DH + 1)[:, :, 0:DH]

        def proj_vaug(w_tiles, actT, vaug_tiles):
            for mt, p in proj_nat_pair(w_tiles, actT):
                dst = vaug_dst(vaug_tiles[mt])
                nc.scalar.activation(dst, p[:], AF.Copy, scale=1.0 / WS)

        def emit_kv(kT_tiles, vaug_tiles, ag_in, halfk=None):
            """kT pair tiles + vaug tiles -> ag DRAM buffer (bf16-typed)."""
            hps = range(HC) if halfk is None else range(4 * halfk, 4 * halfk + 4)
            for i, hp in enumerate(hps):
                mp, half = hp // 2, hp % 2
                src = kT_tiles[mp][:, half * T:(half + 1) * T].bitcast(BF16)
                nc.sync.dma_start(
                    ag_in[i * KT_CH:(i + 1) * KT_CH]
                    .rearrange("(p f) -> p f", p=128), src)
            ktl = len(list(hps)) * KT_CH
            for mt in range(TC):
                src = vaug_tiles[mt][:] if halfk is None else \
                    vaug_tiles[mt][:, halfk * (VA_ROW // 2):
                                   (halfk + 1) * (VA_ROW // 2)]
                w = src.free_size() // 2
                nc.sync.dma_start(
                    ag_in[ktl + mt * (128 * w):ktl + (mt + 1) * (128 * w)]
                    .rearrange("(p f) -> p f", p=128), src.bitcast(BF16))

        # =====================================================================
        # attention inner loop
        # =====================================================================
        def prepare_attention(kt_src, vt_src):
            pre = {"kts": {}, "vts": {}}

            def load_kt(hp):
                kt = ktp.tile([128, 2 * T], E4, name="kt", tag="kt")
                for sl in range(2):
                    nc.sync.dma_start(
                        kt[:, sl * T:(sl + 1) * T].bitcast(BF16),
                        kt_src(hp, sl))
                pre["kts"][hp] = kt

            def load_vts(hpp):
                lst = []
                for tkp in range(HC // 2):
                    vt = vtp.tile([128, 2 * VA_P], E4, name="vt", tag="vt")
                    for s in range(2):
                        tk = 2 * tkp + s
                        sl, ro = tk // TC, (tk % TC) * 128
                        nc.sync.dma_start(
                            vt[:, s * VA_P:s * VA_P + VA_G].bitcast(BF16),
                            vt_src(hpp, sl, ro))
                    lst.append(vt[:].rearrange("p (two f) -> p two f", two=2))
                pre["vts"][hpp] = lst

            pre["load_kt"] = load_kt
            pre["load_vts"] = load_vts
            load_kt(0)
            load_kt(1)
            load_vts(0)
            return pre

        def emit_attention(qT_pairs, pre, wo_tiles, resid_tiles,
                           out_tag, per_mt_hook=None):
            """64 flat iterations (8 hp x 8 tk); PV every other iteration via
            fp8 DoubleRow over key-tile pairs."""
            NIT = HC * HC  # 64
            oT_pairs = [e4p.tile([128, 2 * T], E4, name="oT", tag="e4pair")
                        for _ in range(HP)]
            kts = pre["kts"]
            vts = pre["vts"]
            load_kt = pre["load_kt"]
            load_vts = pre["load_vts"]
            accs = {}
            pairs = [None] * NIT
            pps = {}
            tails = {}

            def stage_scores(it):
                hp, tk = it // HC, it % HC
                if tk == 0:
                    if hp + 2 < HC:
                        load_kt(hp + 2)
                    accs[hp] = (ps.tile([128, 512], F32, name="oacc", tag="ps"),
                                ps.tile([128, 512], F32, name="oacc", tag="ps"))
                if tk == 4 and hp % 2 == 1 and hp // 2 + 1 < 4:
                    load_vts(hp // 2 + 1)
                pair = pspair.tile([128, 1024], F32, name="spair", tag="sp")
                kt = kts[hp]
                mp, half = hp // 2, hp % 2
                qT = qT_pairs[mp]
                nc.tensor.matmul(pair[:, 0:512],
                                 kt[0:64, tk * 128:(tk + 1) * 128],
                                 qT[0:64, half * T:(half + 1) * T],
                                 start=True, stop=True, tile_position=(0, 0))
                nc.tensor.matmul(pair[:, 512:1024],
                                 kt[64:128, tk * 128:(tk + 1) * 128],
                                 qT[64:128, half * T:(half + 1) * T],
                                 start=True, stop=True, tile_position=(64, 0))
                pairs[it] = pair

            ESP = 640   # scalar handles [0:ESP], DVE [ESP:1024] of each tile

            def stage_exp(it):
                tk = it % HC
                if tk % 2 == 0:
                    pp = ppp.tile([128, 2 * 1024], E5, name="pp", tag="pp")
                    pps[it // 2] = pp
                else:
                    pp = pps[it // 2]
                base = (tk % 2) * 1024
                nc.scalar.activation(pp[:, base:base + ESP],
                                     pairs[it][:, 0:ESP], AF.Exp,
                                     scale=1.0 / A5)
                nc.vector.tensor_scalar(
                    pp[:, base + ESP:base + 1024].bitcast(I8),
                    pairs[it][:, ESP:1024], B5, None, op0=OP.add)
                pairs[it] = None

            def stage_pv(it, step):
                hp, tk = it // HC, it % HC
                if tk % 2 == 0:
                    return
                hpp, i = hp // 2, hp % 2
                tkp = tk // 2
                vt = vts[hpp][tkp]
                pp3 = pps[it // 2][:].rearrange("p (two f) -> p two f", two=2)
                oa, ob = accs[hp]
                for hi in range(2):
                    head = 2 * i + hi
                    dst = oa if hi == 0 else ob
                    nc.tensor.matmul(
                        dst[0:DH + 1, :],
                        vt[:, :, head * (DH + 1):(head + 1) * (DH + 1)],
                        pp3[:, :, hi * 512:(hi + 1) * 512],
                        start=(tkp == 0), stop=(tkp == 3), perf_mode=DR)
                pps[it // 2] = None
                if tk == HC - 1:
                    schedule_tail(hp, step)

            def schedule_tail(hp, step):
                oa, ob = accs.pop(hp)
                mp, half = hp // 2, hp % 2
                st = {}

                def t0():
                    st["den"] = rowp.tile([1, 2 * T], F32, name="den", tag="den")
                    nc.scalar.copy(st["den"][:, 0:T], oa[DH:DH + 1, :])
                    nc.vector.tensor_copy(st["den"][:, T:2 * T], ob[DH:DH + 1, :])

                def t1():
                    st["recip"] = rowp.tile([1, 2 * T], F32, name="recip", tag="recip")
                    nc.vector.reciprocal_approx_fast(st["recip"][:], st["den"][:])

                def t2():
                    st["recr"] = rowp.tile([1, 2 * T], BF16, name="recr", tag="recr")
                    nc.scalar.copy(st["recr"][:], st["recip"][:])

                def t3():
                    st["bcs"] = bcsp.tile([64, 2 * T], BF16, name="bcs", tag="bcs")
                    nc.gpsimd.partition_broadcast(st["bcs"][:], st["recr"][:])

                def t4():
                    oT = oT_pairs[mp]
                    nc.vector.tensor_mul(oT[0:64, half * T:(half + 1) * T],
                                         oa[0:64, :], st["bcs"][:, 0:T])
                    nc.vector.tensor_mul(oT[64:128, half * T:(half + 1) * T],
                                         ob[0:64, :], st["bcs"][:, T:2 * T])

                for off, fn in ((1, t0), (2, t1), (3, t2), (4, t3), (6, t4)):
                    tails.setdefault(step + off, []).append(fn)

            for step in range(NIT + 8):
                if step < NIT:
                    stage_scores(step)
                if 1 <= step <= NIT:
                    stage_exp(step - 1)
                if 2 <= step <= NIT + 1:
                    stage_pv(step - 2, step)
                for fn in tails.pop(step, ()):
                    fn()

            # oT pair cols [0:T] = hp even chunk = H rows [256mp:256mp+128]
            oT3 = [o[:].rearrange("p (two f) -> p two f", two=2)
                   for o in oT_pairs]
            out_tiles = [full.tile([128, H], F32, name=out_tag, tag="big")
                         for _ in range(TC)]
            for mt, p in proj_nat_pair(wo_tiles, oT3):
                nc.vector.scalar_tensor_tensor(
                    out_tiles[mt][:], p[:], 1.0 / (16.0 * WS),
                    resid_tiles[mt][:], op0=OP.mult, op1=OP.add)
                if per_mt_hook is not None:
                    per_mt_hook(mt, out_tiles[mt])
            return out_tiles

        # =====================================================================
        # Phase 1: silu -> siluT -> ada MMs (PE) while hn/xln LN chains (DVE)
        # =====================================================================
        with nc.named_scope("p1_ada"):
            silu = []
            for mt in range(TC):
                sl_t = halfp.tile([128, H], BF16, name="silu", tag="bighalf")
                nc.vector.tensor_add(sl_t[:], t_sb[mt][:], h_sb[mt][:])
                nc.scalar.activation(sl_t[:], sl_t[:], AF.Silu)
                silu.append(sl_t)
            siluT, _ = transpose_act(silu, "siluT")

            shift_sb = [bmod.tile([128, H], BF16, name="shift", tag="mod") for _ in range(TC)]
            scale1_sb = [bmod.tile([128, H], BF16, name="scale1", tag="mod") for _ in range(TC)]
            for mt, p in proj_nat_pair(ada_tiles0, siluT):
                nc.scalar.activation(shift_sb[mt][:], p[:], AF.Copy,
                                     scale=1.0 / WS, bias=0.0)
            ada_tiles1 = load_weight("ada_w", half=1)
            for mt, p in proj_nat_pair(ada_tiles1, siluT):
                nc.scalar.activation(scale1_sb[mt][:], p[:], AF.Copy,
                                     scale=1.0 / WS, bias=1.0)

        # =====================================================================
        # Phase 1b: cross K/V matmuls (fills PE while xln/mods run on DVE);
        # the AllGather itself is issued later, after the self-AG halves.
        # =====================================================================
        def kv_evict(p, o, mp):
            if mp % 2 == 0:
                nc.scalar.activation(o[:], p[:], AF.Copy, scale=1.0 / WS)
            else:
                nc.vector.tensor_scalar(o[:], p[:], 1.0 / WS, None, op0=OP.mult)

        def kv_evict(p, o, mp):
            nc.scalar.activation(o[:], p[:], AF.Copy, scale=1.0 / WS)

        wck = load_weight("Wck")
        wcv = load_weight("Wcv")
        with nc.named_scope("p1b_crosskv"):
            hn = layernorm(h_sb, BF16, "hn", nc.vector)
            hnT, _ = transpose_act(hn, "hnT")
            kcT = proj_T_pair(wck, hnT, "kcT", kv_evict)
            vaug_c = make_vaug_tiles()
            proj_vaug(wcv, hnT, vaug_c)



        # =====================================================================
        # Phase 2b: hidden_in = xln * (1+scale) + shift; hinT
        # =====================================================================
        with nc.named_scope("p2b_hin"):
            xln = layernorm(x_sb, BF16, "xln", nc.vector)
            hin = []
            for mt in range(TC):
                nc.vector.tensor_mul(xln[mt][:], xln[mt][:], scale1_sb[mt][:])
                nc.vector.tensor_add(xln[mt][:], xln[mt][:], shift_sb[mt][:])
                hin.append(xln[mt])
            hinT, _ = transpose_act(hin, "hinT", engines=("s", "s"))


        # ag source maps -----------------------------------------------------
        def kt_src_s(hp, sl):
            half, hpl = hp // 4, hp % 4
            return ag_so[half][sl * HALF_LEN + hpl * KT_CH:
                               sl * HALF_LEN + (hpl + 1) * KT_CH] \
                .rearrange("(p f) -> p f", p=128)

        def vt_src_s(hpp, sl, ro):
            half, hl = hpp // 2, hpp % 2
            v = ag_so[half][sl * HALF_LEN + HALF_KT:
                            sl * HALF_LEN + HALF_KT + HALF_VA] \
                .rearrange("(tt f) -> tt f", tt=T)
            return v[ro:ro + 128, hl * (VA_G // 2):(hl + 1) * (VA_G // 2)]

        def kt_src_c(hp, sl):
            return ag_out_c[sl * RANK_LEN + hp * KT_CH:
                            sl * RANK_LEN + (hp + 1) * KT_CH] \
                .rearrange("(p f) -> p f", p=128)

        def vt_src_c(hpp, sl, ro):
            v = ag_out_c[sl * RANK_LEN + KT_LEN:
                         sl * RANK_LEN + KT_LEN + VA_LEN] \
                .rearrange("(tt f) -> tt f", tt=T)
            return v[ro:ro + 128, hpp * (VA_G // 2):(hpp + 1) * (VA_G // 2)]

        wsk = load_weight("Wsk")
        wsv = load_weight("Wsv")
        # =====================================================================
        # Phase 3: self K/V halves + AllGather(self)
        # =====================================================================
        with nc.named_scope("p3_selfkv"):
            vaug_s = make_vaug_tiles()
            ksT = [None] * HP
            ag_s = (ag_in_s0, ag_in_s1)
            ag_so = (ag_out_s0, ag_out_s1)
            for halfk in range(2):
                part = proj_T_pair(wsk, hinT, "ksT", kv_evict,
                                   mo_pairs=[2 * halfk, 2 * halfk + 1])
                ksT[2 * halfk] = part[0]
                ksT[2 * halfk + 1] = part[1]
                for mt, p in proj_nat_pair(wsv, hinT, n_list=(halfk,)):
                    dst = vaug_dst(vaug_s[mt], halfk=halfk)
                    src = p[:, halfk * 512:(halfk + 1) * 512]
                    if mt % 2 == 0:
                        nc.scalar.activation(dst, src, AF.Copy, scale=1.0 / WS)
                    else:
                        nc.vector.tensor_scalar(dst, src, 1.0 / WS, None, op0=OP.mult)
                agi = ag_s[halfk]
                emit_kv(ksT, vaug_s, agi, halfk=halfk)
                nc.gpsimd.collective_compute(
                    "AllGather", OP.bypass,
                    replica_groups=[[0, 1], [2, 3], [4, 5], [6, 7]],
                    ins=[agi.opt()], outs=[ag_so[halfk].opt()])
                if halfk == 0:
                    pre_s = prepare_attention(kt_src_s, vt_src_s)

        wsq = load_weight("Wsq")
        wso = load_weight("Wso")

        def q_evict(bc):
            def ev(p, o, mp):
                nc.vector.tensor_mul(o[:], p[:], bc[:])
            return ev

        with nc.named_scope("p4_qself"):
            em_bc = mask_bcast("em", "em", 0.125 * A5 / WS)
            qsT = proj_T_pair(wsq, hinT, "qsT", q_evict(em_bc))

        # =====================================================================
        # Phase 4: self attention; cross q
        # =====================================================================
        wcq = load_weight("Wcq")
        wco = load_weight("Wco")
        with nc.named_scope("p5_selfattn"):
            ln2o = [None] * TC

            def ln2_hook(mt, tile_):
                ln2o[mt] = layernorm_tile(tile_, BF16, "ln2o", nc.vector)

            hidden_in = emit_attention(qsT, pre_s, wso,
                                       x_sb, "res1", per_mt_hook=ln2_hook)

        with nc.named_scope("p2_crossag"):
            emit_kv(kcT, vaug_c, ag_in_c)
            nc.gpsimd.collective_compute(
                "AllGather", OP.bypass,
                replica_groups=[[0, 1], [2, 3], [4, 5], [6, 7]],
                ins=[ag_in_c.opt()], outs=[ag_out_c.opt()])
        w1 = load_weight("ffn_w1")
        pre_c = prepare_attention(kt_src_c, vt_src_c)
        with nc.named_scope("p6_ln2q"):
            mk_bc = mask_bcast("mk", "mk", 0.125 * A5 / WS)
            ln2T, _ = transpose_act(ln2o, "ln2T", engines=("s", "s"))
            qcT = proj_T_pair(wcq, ln2T, "qcT", q_evict(mk_bc))

        # =====================================================================
        # Phase 5: cross attention -> hidden_mid
        # =====================================================================
        w2 = load_weight("ffn_w2")
        with nc.named_scope("p7_crossattn"):
            ln3o = [None] * TC

            def ln3_hook(mt, tile_):
                ln3o[mt] = layernorm_tile(tile_, F32, "ln3o", nc.vector,
                                          out_pool=full)

            hidden_mid = emit_attention(qcT, pre_c, wco,
                                        hidden_in, "hmid", per_mt_hook=ln3_hook)

        # =====================================================================
        # Phase 6: FFN + final LN + output (mt-pipelined tail)
        # =====================================================================
        with nc.named_scope("p8_ffn"):
            hoT, _ = transpose_act(ln3o, "hoT", fp32=True, engines=("s", "s"))
            midT_raw = []
            for mp in range(HP):
                p = pspair.tile([128, 1024], F32, name="spair", tag="sp")
                for half in range(2):
                    mo = 2 * mp + half
                    for j in range(4):
                        nc.tensor.matmul(
                            p[:, half * 512:(half + 1) * 512],
                            w1[j][:, :, mo * 128:(mo + 1) * 128],
                            hoT[j][:],
                            start=(j == 0), stop=(j == 3), perf_mode=DR)
                o = e4p.tile([128, 2 * T], E4, name="midT", tag="e4pair")
                if mp % 2 == 0:
                    nc.scalar.activation(o[:], p[:], AF.Relu, scale=1.0 / WS)
                else:
                    nc.vector.tensor_scalar(o[:], p[:], 1.0 / WS, 0.0,
                                            op0=OP.mult, op1=OP.max)
                midT_raw.append(o)
            midT = [o[:].rearrange("p (two f) -> p two f", two=2)
                    for o in midT_raw]

            for mt, p in proj_nat_pair(w2, midT):
                ffres = full.tile([128, H], F32, name="ffres", tag="big")
                nc.vector.scalar_tensor_tensor(
                    ffres[:], p[:], 1.0 / WS,
                    ln3o[mt][:], op0=OP.mult, op1=OP.add)
                lnfo = layernorm_tile(ffres, F32, "lnfo", nc.vector,
                                      out_pool=full)
                nc.vector.tensor_add(lnfo[:], lnfo[:], hidden_mid[mt][:])
                nc.sync.dma_start(out_ext.ap()[mt * 128:(mt + 1) * 128, :],
                                  lnfo[:])


def _bf16(a):
    import ml_dtypes
    return np.asarray(a, np.float32).astype(ml_dtypes.bfloat16)


def _e4(a):
    import ml_dtypes
    return np.clip(np.asarray(a, np.float32), -240, 240).astype(
        ml_dtypes.float8_e4m3)


def _pack_w(w):
    """[K, N] -> [512, 2N] e4m3 pair-packed, prescaled x32."""
    w = np.asarray(w, np.float32) * WS
    K, N = w.shape
    wp = w.reshape(4, 2, 128, N).transpose(0, 2, 1, 3).reshape(512, 2 * N)
    return np.ascontiguousarray(_e4(wp))


def kernel(**inputs):
    global LAST_RESULT
    if "nc" not in _BUILD_CACHE:
        _BUILD_CACHE["nc"] = _build()
    nc = _BUILD_CACHE["nc"]

    x = np.asarray(inputs["x"], np.float32)
    h = np.asarray(inputs["h"], np.float32)
    t = np.asarray(inputs["t"], np.float32)
    em = np.asarray(inputs["extent_mask"], np.int32)
    mk = np.asarray(inputs["mask"], np.int32)

    common = {}
    for nm in W_NAMES + ["ada_w"]:
        common[nm] = _pack_w(inputs[nm])
    common["c_ones"] = _bf16(np.ones((1, T)))
    common["c_ident"] = np.eye(128, dtype=np.float32)
    common["c_identb"] = _bf16(np.eye(128))

    in_maps = []
    for c in range(NC):
        b, half = c // 2, c % 2
        s0 = half * T
        m = dict(common)
        m["x"] = np.ascontiguousarray(_bf16(x[b, s0:s0 + T]))
        m["h"] = np.ascontiguousarray(_bf16(h[b, s0:s0 + T]))
        m["t"] = np.ascontiguousarray(_bf16(t[b, s0:s0 + T]))
        m["em"] = np.ascontiguousarray(em[b, s0:s0 + T].reshape(1, T))
        m["mk"] = np.ascontiguousarray(mk[b, s0:s0 + T].reshape(1, T))
        in_maps.append(m)

    trace = bool(os.environ.get("BASS_TRACE_KERNEL"))
    if trace:
        _install_ntff_hook()
    try:
        res = bass_utils.run_bass_kernel_spmd(
            nc, in_maps, core_ids=list(range(NC)), trace=trace)
    except Exception:
        import time
        time.sleep(20)
        res = bass_utils.run_bass_kernel_spmd(
            nc, in_maps, core_ids=list(range(NC)), trace=trace)
    LAST_RESULT = res

    out = np.empty((B, S, H), np.float32)
    for c in range(NC):
        b, half = c // 2, c % 2
        out[b, half * T:(half + 1) * T] = res.results[c]["out"]
    return out


def _install_ntff_hook():
    import sys, types
    if 'antenv.axon_hooks' in sys.modules:
        return
    mod = types.ModuleType("antenv.axon_hooks")
    mod._hook = None
    def set_axon_ntff_profile_hook(h): mod._hook = h
    def get_axon_ntff_profile_hook(): return mod._hook
    mod.set_axon_ntff_profile_hook = set_axon_ntff_profile_hook
    mod.get_axon_ntff_profile_hook = get_axon_ntff_profile_hook
    sys.modules['antenv.axon_hooks'] = mod
    import antenv
    antenv.axon_hooks = mod
    try:
        from trn_agent_boot.trn_boot import _ntff_profile_via_ctypes
        mod.set_axon_ntff_profile_hook(
            _ntff_profile_via_ctypes('/opt/axon/libaxon_pjrt.so'))
    except Exception:
        pass


# revision 21
# speedup vs baseline: 1.1618x; 1.0043x over previous
"""AdaLN transformer block on 8 TRN2 NeuronCores (Bass/Tile), fp8 edition.

Sharding: 4096 tokens (B*S) split 8 ways -> 512 tokens/core; cores (2b, 2b+1)
own batch element b. Attention needs full-sequence K/V per batch element:
pairwise AllGather (replica groups [[0,1],[2,3],[4,5],[6,7]]) of fp8 K^T /
V(+1/16 col).

Precision: all projections run in fp8 e4m3 with DoubleRow perf mode (two
128-row contraction chunks per matmul -> 2x PE throughput; weights are
host-prescaled x32 and pair-packed [128,2,N]). Scores use e4m3 qT/kT with the
softmax exp prescale A5=4/ln2 folded into the q eviction; probabilities are
e5m2 via ScalarE exact exp (even key tiles) and a DVE int8 Schraudolph trick
(odd key tiles, round-to-nearest, tuned B5). PV runs fp8 DoubleRow over
key-tile pairs; the V ones-column is 1/16 so the reciprocal fold lands oT in
e4m3 range (x16). The f32 spine (x residual, LN stats, PSUM accumulation,
softmax denominators) keeps total rel err ~1e-2 (tolerance 2e-2).
"""
import os
import math
import numpy as np

import concourse.bass as bass
import concourse.bacc as bacc
import concourse.tile as tile
import concourse.mybir as mybir
from concourse import bass_utils

B, S, H, NH = 4, 1024, 1024, 16
DH = H // NH               # 64
EPS = 1e-5
NC = 8
T = (B * S) // NC          # 512 tokens per core
TC = T // 128              # 4
HC = H // 128              # 8
HP = HC // 2               # 4 pair tiles per H-contraction

VA_G = 4 * (DH + 1)        # 260: one 4-head group's v_aug row chunk (fp8)
VA_ROW = 4 * VA_G          # 1040: full v_aug row (16 heads)
VA_P = 272                 # padded pair-slot stride in vt tiles (mult of 16)
KT_CH = 128 * T // 2       # 32768: one [128,T] fp8 kT chunk in bf16 elems
KT_LEN = HC * KT_CH        # 262144
VA_LEN = T * VA_ROW // 2   # 266240
RANK_LEN = KT_LEN + VA_LEN
HALF_KT = 4 * KT_CH
HALF_VA = T * (VA_ROW // 2) // 2
HALF_LEN = HALF_KT + HALF_VA

F32 = mybir.dt.float32
BF16 = mybir.dt.bfloat16
E4 = mybir.dt.float8e4
E5 = mybir.dt.float8e5
I8 = mybir.dt.int8
I32 = mybir.dt.int32
AF = mybir.ActivationFunctionType
OP = mybir.AluOpType
DR = mybir.MatmulPerfMode.DoubleRow

WS = 32.0                  # host weight prescale (power of 2)
A5 = 4.0 / math.log(2.0)   # e5m2 Schraudolph scale (folded into q eviction)
B5 = 59.76                 # tuned for round-to-nearest f32->int8
ONE_C = 1.0 / 16.0         # v_aug ones column; recip fold scales oT x16

W_NAMES = ["Wsq", "Wsk", "Wsv", "Wso", "Wcq", "Wck", "Wcv", "Wco",
           "ffn_w1", "ffn_w2"]

LAST_RESULT = None
_BUILD_CACHE = {}


def _build():
    nc = bacc.Bacc("TRN2", target_bir_lowering=False, debug=False,
                   num_devices=NC)
    ext = {}
    for nm in ["x", "h", "t"]:
        ext[nm] = nc.dram_tensor(nm, [T, H], BF16, kind="ExternalInput")
    ext["em"] = nc.dram_tensor("em", [1, T], I32, kind="ExternalInput")
    ext["mk"] = nc.dram_tensor("mk", [1, T], I32, kind="ExternalInput")
    for nm in W_NAMES:
        ext[nm] = nc.dram_tensor(nm, [4 * 128, 2 * H], E4, kind="ExternalInput")
    ext["ada_w"] = nc.dram_tensor("ada_w", [4 * 128, 2 * 2 * H], E4,
                                  kind="ExternalInput")
    ext["c_ones"] = nc.dram_tensor("c_ones", [1, T], BF16, kind="ExternalInput")
    ext["c_ident"] = nc.dram_tensor("c_ident", [128, 128], F32, kind="ExternalInput")
    ext["c_identb"] = nc.dram_tensor("c_identb", [128, 128], BF16, kind="ExternalInput")
    out_ext = nc.dram_tensor("out", [T, H], F32, kind="ExternalOutput")

    with tile.TileContext(nc) as tc:
        _emit(nc, tc, ext, out_ext)
    nc.compile()
    return nc


def _emit(nc, tc, ext, out_ext):
    import contextlib
    ctx = contextlib.ExitStack()
    with ctx:
        full = ctx.enter_context(tc.tile_pool(name="full", bufs=13))
        halfp = ctx.enter_context(tc.tile_pool(name="halfp", bufs=16))
        e4p = ctx.enter_context(tc.tile_pool(name="e4p", bufs=14))
        wpool = ctx.enter_context(tc.tile_pool(name="wpool", bufs=16))
        bmod = ctx.enter_context(tc.tile_pool(name="bmod", bufs=8))
        ktp = ctx.enter_context(tc.tile_pool(name="ktp", bufs=3))
        vtp = ctx.enter_context(tc.tile_pool(name="vtp", bufs=8))
        ppp = ctx.enter_context(tc.tile_pool(name="ppp", bufs=3))
        bcsp = ctx.enter_context(tc.tile_pool(name="bcsp", bufs=2))
        vaugp = ctx.enter_context(tc.tile_pool(name="vaugp", bufs=4))
        smalls = ctx.enter_context(tc.tile_pool(name="smalls", bufs=1))
        stat = ctx.enter_context(tc.tile_pool(name="stat", bufs=8))
        rowp = ctx.enter_context(tc.tile_pool(name="rowp", bufs=2))
        ps = ctx.enter_context(tc.tile_pool(name="ps", bufs=4, space="PSUM"))
        pspair = ctx.enter_context(tc.tile_pool(name="pspair", bufs=2, space="PSUM"))
        dram = ctx.enter_context(tc.tile_pool(name="dram", bufs=1, space="DRAM"))

        # ---------------- constants ----------------
        ones = smalls.tile([1, T], BF16, name="ones", tag="ones")
        nc.sync.dma_start(ones[:], ext["c_ones"].ap())
        ident = smalls.tile([128, 128], F32, name="ident", tag="ident")
        nc.sync.dma_start(ident[:], ext["c_ident"].ap())
        identb = smalls.tile([128, 128], BF16, name="identb", tag="identb")
        nc.sync.dma_start(identb[:], ext["c_identb"].ap())
        eps_t = smalls.tile([128, 1], F32, name="eps_t", tag="eps_t")
        nc.vector.memset(eps_t[:], EPS)
        # HAM warmup: keep PE busy during the initial input DMA.
        wu = ps.tile([128, 512], F32, name="wu", tag="ps")
        wu_src = ident[:].bitcast(BF16)
        for _ in range(30):
            nc.tensor.matmul(wu[:, 0:256], wu_src[:, 0:128], wu_src[:],
                             start=True, stop=True)

        # skew absorber: tiny AllGather at t=0; DVE syncs on it so all cores
        # align before the heavy phases (later collectives then see ~0 skew).
        dummy_in = dram.tile([128], BF16, name="dummy_in", tag="dummy_in")
        dummy_out = dram.tile([256], BF16, name="dummy_out", tag="dummy_out")
        dsb = smalls.tile([1, 128], BF16, name="dsb", tag="dsb")
        nc.vector.memset(dsb[:], 0.0)
        nc.sync.dma_start(dummy_in.rearrange("(p f) -> p f", p=1), dsb[:])
        nc.gpsimd.collective_compute(
            "AllGather", OP.bypass,
            replica_groups=[[0, 1], [2, 3], [4, 5], [6, 7]],
            ins=[dummy_in.opt()], outs=[dummy_out.opt()])
        dsb2 = smalls.tile([1, 256], BF16, name="dsb2", tag="dsb2")
        nc.sync.dma_start(dsb2[:], dummy_out.rearrange("(p f) -> p f", p=1))
        dsb3 = smalls.tile([1, 256], BF16, name="dsb3", tag="dsb3")
        nc.vector.tensor_copy(dsb3[:], dsb2[:])

        # ---------------- input loads (h first: needed earliest) ----------
        h_sb, t_sb, x_sb = [], [], []
        for mt in range(TC):
            th = halfp.tile([128, H], BF16, name="h", tag="bighalf")
            nc.sync.dma_start(th[:], ext["h"].ap()[mt * 128:(mt + 1) * 128, :])
            h_sb.append(th)

        def load_weight(nm, half=None):
            """fp8 pair-packed weight: 4 tiles [128, 2, N] (rearranged APs)."""
            n = 2 * H if nm == "ada_w" else H
            if half is not None:
                n = n // 2
            tiles = []
            for j in range(4):
                t_ = wpool.tile([128, 2 * n], E4, name="w_" + nm, tag="w")
                if half is not None:
                    t3 = t_[:].rearrange("p (two f) -> p two f", two=2)
                    for s in range(2):
                        nc.sync.dma_start(
                            t3[:, s, :],
                            ext[nm].ap()[j * 128:(j + 1) * 128,
                                         s * 2 * H + half * n:
                                         s * 2 * H + (half + 1) * n])
                else:
                    nc.sync.dma_start(t_[:], ext[nm].ap()[j * 128:(j + 1) * 128, :])
                tiles.append(t_[:].rearrange("p (two f) -> p two f", two=2))
            return tiles

        for mt in range(TC):
            tt = halfp.tile([128, H], BF16, name="tin", tag="bighalf")
            nc.sync.dma_start(tt[:], ext["t"].ap()[mt * 128:(mt + 1) * 128, :])
            t_sb.append(tt)
        for mt in range(TC):
            tx = halfp.tile([128, H], BF16, name="x", tag="bighalf")
            nc.sync.dma_start(tx[:], ext["x"].ap()[mt * 128:(mt + 1) * 128, :])
            x_sb.append(tx)
        ada_tiles0 = load_weight("ada_w", half=0)

        def mask_bcast(name, tagn, scale):
            """[128, 2T] bf16 broadcast of mask*scale (same mask both halves)."""
            mi = smalls.tile([1, T], I32, name=tagn + "_i", tag=tagn + "_i")
            nc.sync.dma_start(mi[:], ext[name].ap())
            mf = smalls.tile([1, T], F32, name=tagn + "_f", tag=tagn + "_f")
            nc.vector.tensor_copy(mf[:], mi[:])
            mr = smalls.tile([1, T], BF16, name=tagn + "_r", tag=tagn + "_r")
            nc.vector.tensor_scalar_mul(mr[:], mf[:], scale)
            bc = smalls.tile([128, 2 * T], BF16, name=tagn + "_bc", tag=tagn + "_bc")
            p = pspair.tile([128, 1024], F32, name="spair", tag="sp")
            nc.tensor.matmul(p[:, 0:512], ones[:, 0:128], mr[:], start=True, stop=True)
            nc.tensor.matmul(p[:, 512:1024], ones[:, 0:128], mr[:], start=True, stop=True)
            nc.vector.tensor_copy(bc[:], p[:])
            return bc

        ag_in_s0 = dram.tile([HALF_LEN], BF16, name="agins0", tag="agins0")
        ag_out_s0 = dram.tile([2 * HALF_LEN], BF16, name="agouts0", tag="agouts0")
        ag_in_s1 = dram.tile([HALF_LEN], BF16, name="agins1", tag="agins1")
        ag_out_s1 = dram.tile([2 * HALF_LEN], BF16, name="agouts1", tag="agouts1")
        ag_in_c = dram.tile([RANK_LEN], BF16, name="aginc", tag="aginc")
        ag_out_c = dram.tile([2 * RANK_LEN], BF16, name="agoutc", tag="agoutc")

        # ---------------- helpers ----------------
        def layernorm_tile(src, out_dt, out_tag, apply_engine, out_pool=None):
            st = stat.tile([128, 12], F32, name="lnstat", tag="lnstat")
            nc.vector.bn_stats(st[:, 0:6], src[:, 0:512])
            nc.vector.bn_stats(st[:, 6:12], src[:, 512:1024])
            ag = stat.tile([128, 2], F32, name="lnag", tag="lnag")
            nc.vector.bn_aggr(ag[:], st[:])
            sd = stat.tile([128, 1], F32, name="lnsd", tag="lnsd")
            nc.scalar.activation(sd[:], ag[:, 1:2], AF.Sqrt, bias=eps_t[:])
            rstd = stat.tile([128, 1], F32, name="lnrstd", tag="lnrstd")
            nc.vector.reciprocal(rstd[:], sd[:])
            pool = out_pool or (full if out_dt == F32 else halfp)
            o = pool.tile([128, H], out_dt, name=out_tag,
                          tag="big" if out_dt == F32 else "bighalf")
            apply_engine.tensor_scalar(o[:], src[:], ag[:, 0:1],
                                       rstd[:], op0=OP.subtract, op1=OP.mult)
            return o

        def layernorm(src_tiles, out_dt, out_tag, apply_engine):
            return [layernorm_tile(src_tiles[mt], out_dt, out_tag, apply_engine)
                    for mt in range(TC)]

        def transpose_act(src_tiles, out_tag, fp32=False, engines=("v", "s")):
            """natural [T,H] tiles -> 4 e4m3 pair tiles [128, 2T]
            (pair j: cols [0:T]=H-chunk 2j, [T:2T]=chunk 2j+1).
            bf16 sources pack 4 transposed chunks per [128,1024]-f32 psum via
            a bf16 bitcast view; f32 sources use 2 chunks per psum."""
            out_tiles = []
            if not fp32:
                for hg in range(2):
                    pt = pspair.tile([128, 1024], F32, name="spair", tag="sp")
                    ptb = pt[:].bitcast(BF16)   # [128, 2048] bf16
                    for mt in range(TC):
                        for k in range(4):
                            hh = hg * 4 + k
                            nc.tensor.transpose(
                                ptb[:, k * 512 + mt * 128: k * 512 + (mt + 1) * 128],
                                src_tiles[mt][:, hh * 128:(hh + 1) * 128],
                                identb[:])
                    for j2 in range(2):
                        o = e4p.tile([128, 2 * T], E4, name=out_tag, tag="e4pair")
                        if engines[j2] == "v":
                            nc.vector.tensor_copy(o[:], ptb[:, j2 * 1024:(j2 + 1) * 1024])
                        else:
                            nc.scalar.copy(o[:], ptb[:, j2 * 1024:(j2 + 1) * 1024])
                        out_tiles.append(o)
            else:
                for hg in range(4):
                    pt = pspair.tile([128, 1024], F32, name="spair", tag="sp")
                    for mt in range(TC):
                        for k in range(2):
                            hh = hg * 2 + k
                            nc.tensor.transpose(
                                pt[:, k * 512 + mt * 128: k * 512 + (mt + 1) * 128],
                                src_tiles[mt][:, hh * 128:(hh + 1) * 128],
                                ident[:])
                    o = e4p.tile([128, 2 * T], E4, name=out_tag, tag="e4pair")
                    if engines[hg % 2] == "v":
                        nc.vector.tensor_copy(o[:], pt[:])
                    else:
                        nc.scalar.copy(o[:], pt[:])
                    out_tiles.append(o)
            return [o[:].rearrange("p (two f) -> p two f", two=2)
                    for o in out_tiles], out_tiles

        def proj_T_pair(w_tiles, actT, out_tag, evict, mo_pairs=None):
            """(act @ W)^T as raw pair tiles [128, 2T] (cols [0:T]=chunk 2mp).
            evict(pspair, out_tile, mp) writes the FD-1024 eviction."""
            out_tiles = []
            for mp in (mo_pairs if mo_pairs is not None else range(HP)):
                p = pspair.tile([128, 1024], F32, name="spair", tag="sp")
                for half in range(2):
                    mo = 2 * mp + half
                    for j in range(4):
                        nc.tensor.matmul(
                            p[:, half * 512:(half + 1) * 512],
                            w_tiles[j][:, :, mo * 128:(mo + 1) * 128],
                            actT[j][:],
                            start=(j == 0), stop=(j == 3), perf_mode=DR)
                o = e4p.tile([128, 2 * T], E4, name=out_tag, tag="e4pair")
                evict(p, o, mp)
                out_tiles.append(o)
            return out_tiles

        def proj_nat_pair(w_tiles, actT, n_list=(0, 1)):
            """natural-layout projection: yields (mt, pspair [128, 1024])."""
            for mt in range(TC):
                p = pspair.tile([128, 1024], F32, name="spair", tag="sp")
                for n in n_list:
                    for j in range(4):
                        nc.tensor.matmul(
                            p[:, n * 512:(n + 1) * 512],
                            actT[j][:, :, mt * 128:(mt + 1) * 128],
                            w_tiles[j][:, :, n * 512:(n + 1) * 512],
                            start=(j == 0), stop=(j == 3), perf_mode=DR)
                yield mt, p

        def make_vaug_tiles():
            vaug_tiles = []
            for mt in range(TC):
                vt = vaugp.tile([128, VA_ROW], E4, name="vt", tag="vaug")
                nc.vector.memset(vt[:], ONE_C)
                vaug_tiles.append(vt)
            return vaug_tiles

        def vaug_dst(vt, halfk=None):
            src = vt[:] if halfk is None else \
                vt[:, halfk * (VA_ROW // 2):(halfk + 1) * (VA_ROW // 2)]
            return src.rearrange("p (hd c) -> p hd c", c=DH + 1)[:, :, 0:DH]

        def proj_vaug(w_tiles, actT, vaug_tiles):
            for mt, p in proj_nat_pair(w_tiles, actT):
                dst = vaug_dst(vaug_tiles[mt])
                nc.scalar.activation(dst, p[:], AF.Copy, scale=1.0 / WS)

        def emit_kv(kT_tiles, vaug_tiles, ag_in, halfk=None):
            """kT pair tiles + vaug tiles -> ag DRAM buffer (bf16-typed)."""
            hps = range(HC) if halfk is None else range(4 * halfk, 4 * halfk + 4)
            for i, hp in enumerate(hps):
                mp, half = hp // 2, hp % 2
                src = kT_tiles[mp][:, half * T:(half + 1) * T].bitcast(BF16)
                nc.sync.dma_start(
                    ag_in[i * KT_CH:(i + 1) * KT_CH]
                    .rearrange("(p f) -> p f", p=128), src)
            ktl = len(list(hps)) * KT_CH
            for mt in range(TC):
                src = vaug_tiles[mt][:] if halfk is None else \
                    vaug_tiles[mt][:, halfk * (VA_ROW // 2):
                                   (halfk + 1) * (VA_ROW // 2)]
                w = src.free_size() // 2
                nc.sync.dma_start(
                    ag_in[ktl + mt * (128 * w):ktl + (mt + 1) * (128 * w)]
                    .rearrange("(p f) -> p f", p=128), src.bitcast(BF16))

        # =====================================================================
        # attention inner loop
        # =====================================================================
        def prepare_attention(kt_src, vt_src):
            pre = {"kts": {}, "vts": {}}

            def load_kt(hp):
                kt = ktp.tile([128, 2 * T], E4, name="kt", tag="kt")
                for sl in range(2):
                    nc.sync.dma_start(
                        kt[:, sl * T:(sl + 1) * T].bitcast(BF16),
                        kt_src(hp, sl))
                pre["kts"][hp] = kt

            def load_vts(hpp):
                lst = []
                for tkp in range(HC // 2):
                    vt = vtp.tile([128, 2 * VA_P], E4, name="vt", tag="vt")
                    for s in range(2):
                        tk = 2 * tkp + s
                        sl, ro = tk // TC, (tk % TC) * 128
                        nc.sync.dma_start(
                            vt[:, s * VA_P:s * VA_P + VA_G].bitcast(BF16),
                            vt_src(hpp, sl, ro))
                    lst.append(vt[:].rearrange("p (two f) -> p two f", two=2))
                pre["vts"][hpp] = lst

            pre["load_kt"] = load_kt
            pre["load_vts"] = load_vts
            load_kt(0)
            load_kt(1)
            load_vts(0)
            return pre

        def emit_attention(qT_pairs, pre, wo_tiles, resid_tiles,
                           out_tag, per_mt_hook=None):
            """64 flat iterations (8 hp x 8 tk); PV every other iteration via
            fp8 DoubleRow over key-tile pairs."""
            NIT = HC * HC  # 64
            oT_pairs = [e4p.tile([128, 2 * T], E4, name="oT", tag="e4pair")
                        for _ in range(HP)]
            kts = pre["kts"]
            vts = pre["vts"]
            load_kt = pre["load_kt"]
            load_vts = pre["load_vts"]
            accs = {}
            pairs = [None] * NIT
            pps = {}
            tails = {}

            def stage_scores(it):
                hp, tk = it // HC, it % HC
                if tk == 0:
                    if hp + 2 < HC:
                        load_kt(hp + 2)
                    accs[hp] = (ps.tile([128, 512], F32, name="oacc", tag="ps"),
                                ps.tile([128, 512], F32, name="oacc", tag="ps"))
                if tk == 4 and hp % 2 == 1 and hp // 2 + 1 < 4:
                    load_vts(hp // 2 + 1)
                pair = pspair.tile([128, 1024], F32, name="spair", tag="sp")
                kt = kts[hp]
                mp, half = hp // 2, hp % 2
                qT = qT_pairs[mp]
                nc.tensor.matmul(pair[:, 0:512],
                                 kt[0:64, tk * 128:(tk + 1) * 128],
                                 qT[0:64, half * T:(half + 1) * T],
                                 start=True, stop=True, tile_position=(0, 0))
                nc.tensor.matmul(pair[:, 512:1024],
                                 kt[64:128, tk * 128:(tk + 1) * 128],
                                 qT[64:128, half * T:(half + 1) * T],
                                 start=True, stop=True, tile_position=(64, 0))
                pairs[it] = pair

            ESP = 640   # scalar handles [0:ESP], DVE [ESP:1024] of each tile

            def stage_exp(it):
                tk = it % HC
                if tk % 2 == 0:
                    pp = ppp.tile([128, 2 * 1024], E5, name="pp", tag="pp")
                    pps[it // 2] = pp
                else:
                    pp = pps[it // 2]
                base = (tk % 2) * 1024
                nc.scalar.activation(pp[:, base:base + ESP],
                                     pairs[it][:, 0:ESP], AF.Exp,
                                     scale=1.0 / A5)
                nc.vector.tensor_scalar(
                    pp[:, base + ESP:base + 1024].bitcast(I8),
                    pairs[it][:, ESP:1024], B5, None, op0=OP.add)
                pairs[it] = None

            def stage_pv(it, step):
                hp, tk = it // HC, it % HC
                if tk % 2 == 0:
                    return
                hpp, i = hp // 2, hp % 2
                tkp = tk // 2
                vt = vts[hpp][tkp]
                pp3 = pps[it // 2][:].rearrange("p (two f) -> p two f", two=2)
                oa, ob = accs[hp]
                for hi in range(2):
                    head = 2 * i + hi
                    dst = oa if hi == 0 else ob
                    nc.tensor.matmul(
                        dst[0:DH + 1, :],
                        vt[:, :, head * (DH + 1):(head + 1) * (DH + 1)],
                        pp3[:, :, hi * 512:(hi + 1) * 512],
                        start=(tkp == 0), stop=(tkp == 3), perf_mode=DR)
                pps[it // 2] = None
                if tk == HC - 1:
                    schedule_tail(hp, step)

            def schedule_tail(hp, step):
                oa, ob = accs.pop(hp)
                mp, half = hp // 2, hp % 2
                st = {}

                def t0():
                    st["den"] = rowp.tile([1, 2 * T], F32, name="den", tag="den")
                    nc.scalar.copy(st["den"][:, 0:T], oa[DH:DH + 1, :])
                    nc.vector.tensor_copy(st["den"][:, T:2 * T], ob[DH:DH + 1, :])

                def t1():
                    st["recip"] = rowp.tile([1, 2 * T], F32, name="recip", tag="recip")
                    nc.vector.reciprocal_approx_fast(st["recip"][:], st["den"][:])

                def t2():
                    st["recr"] = rowp.tile([1, 2 * T], BF16, name="recr", tag="recr")
                    nc.scalar.copy(st["recr"][:], st["recip"][:])

                def t3():
                    st["bcs"] = bcsp.tile([64, 2 * T], BF16, name="bcs", tag="bcs")
                    nc.gpsimd.partition_broadcast(st["bcs"][:], st["recr"][:])

                def t4():
                    oT = oT_pairs[mp]
                    nc.vector.tensor_mul(oT[0:64, half * T:(half + 1) * T],
                                         oa[0:64, :], st["bcs"][:, 0:T])
                    nc.vector.tensor_mul(oT[64:128, half * T:(half + 1) * T],
                                         ob[0:64, :], st["bcs"][:, T:2 * T])

                for off, fn in ((1, t0), (2, t1), (3, t2), (4, t3), (6, t4)):
                    tails.setdefault(step + off, []).append(fn)

            for step in range(NIT + 8):
                if step < NIT:
                    stage_scores(step)
                if 1 <= step <= NIT:
                    stage_exp(step - 1)
                if 2 <= step <= NIT + 1:
                    stage_pv(step - 2, step)
                for fn in tails.pop(step, ()):
                    fn()

            # oT pair cols [0:T] = hp even chunk = H rows [256mp:256mp+128]
            oT3 = [o[:].rearrange("p (two f) -> p two f", two=2)
                   for o in oT_pairs]
            out_tiles = [full.tile([128, H], F32, name=out_tag, tag="big")
                         for _ in range(TC)]
            for mt, p in proj_nat_pair(wo_tiles, oT3):
                nc.vector.scalar_tensor_tensor(
                    out_tiles[mt][:], p[:], 1.0 / (16.0 * WS),
                    resid_tiles[mt][:], op0=OP.mult, op1=OP.add)
                if per_mt_hook is not None:
                    per_mt_hook(mt, out_tiles[mt])
            return out_tiles

        # =====================================================================
        # Phase 1: silu -> siluT -> ada MMs (PE) while hn/xln LN chains (DVE)
        # =====================================================================
        with nc.named_scope("p1_ada"):
            silu = []
            for mt in range(TC):
                sl_t = halfp.tile([128, H], BF16, name="silu", tag="bighalf")
                nc.vector.tensor_add(sl_t[:], t_sb[mt][:], h_sb[mt][:])
                nc.scalar.activation(sl_t[:], sl_t[:], AF.Silu)
                silu.append(sl_t)
            siluT, _ = transpose_act(silu, "siluT")

            shift_sb = [bmod.tile([128, H], BF16, name="shift", tag="mod") for _ in range(TC)]
            scale1_sb = [bmod.tile([128, H], BF16, name="scale1", tag="mod") for _ in range(TC)]
            for mt, p in proj_nat_pair(ada_tiles0, siluT):
                nc.scalar.activation(shift_sb[mt][:], p[:], AF.Copy,
                                     scale=1.0 / WS, bias=0.0)
            ada_tiles1 = load_weight("ada_w", half=1)
            for mt, p in proj_nat_pair(ada_tiles1, siluT):
                nc.scalar.activation(scale1_sb[mt][:], p[:], AF.Copy,
                                     scale=1.0 / WS, bias=1.0)

        # =====================================================================
        # Phase 2b: hidden_in = xln * (1+scale) + shift; hinT
        # =====================================================================
        with nc.named_scope("p2b_hin"):
            xln = layernorm(x_sb, BF16, "xln", nc.vector)
            hin = []
            for mt in range(TC):
                nc.vector.tensor_mul(xln[mt][:], xln[mt][:], scale1_sb[mt][:])
                nc.vector.tensor_add(xln[mt][:], xln[mt][:], shift_sb[mt][:])
                hin.append(xln[mt])
            hinT, _ = transpose_act(hin, "hinT", engines=("s", "s"))


        # ag source maps -----------------------------------------------------
        def kt_src_s(hp, sl):
            half, hpl = hp // 4, hp % 4
            return ag_so[half][sl * HALF_LEN + hpl * KT_CH:
                               sl * HALF_LEN + (hpl + 1) * KT_CH] \
                .rearrange("(p f) -> p f", p=128)

        def vt_src_s(hpp, sl, ro):
            half, hl = hpp // 2, hpp % 2
            v = ag_so[half][sl * HALF_LEN + HALF_KT:
                            sl * HALF_LEN + HALF_KT + HALF_VA] \
                .rearrange("(tt f) -> tt f", tt=T)
            return v[ro:ro + 128, hl * (VA_G // 2):(hl + 1) * (VA_G // 2)]

        def kt_src_c(hp, sl):
            return ag_out_c[sl * RANK_LEN + hp * KT_CH:
                            sl * RANK_LEN + (hp + 1) * KT_CH] \
                .rearrange("(p f) -> p f", p=128)

        def vt_src_c(hpp, sl, ro):
            v = ag_out_c[sl * RANK_LEN + KT_LEN:
                         sl * RANK_LEN + KT_LEN + VA_LEN] \
                .rearrange("(tt f) -> tt f", tt=T)
            return v[ro:ro + 128, hpp * (VA_G // 2):(hpp + 1) * (VA_G // 2)]

        wsk = load_weight("Wsk")
        wsv = load_weight("Wsv")
        def kv_evict(p, o, mp):
            nc.scalar.activation(o[:], p[:], AF.Copy, scale=1.0 / WS)

        # =====================================================================
        # Phase 3: self K/V halves + AllGather(self)
        # =====================================================================
        with nc.named_scope("p3_selfkv"):
            vaug_s = make_vaug_tiles()
            ksT = [None] * HP
            ag_s = (ag_in_s0, ag_in_s1)
            ag_so = (ag_out_s0, ag_out_s1)
            for halfk in range(2):
                part = proj_T_pair(wsk, hinT, "ksT", kv_evict,
                                   mo_pairs=[2 * halfk, 2 * halfk + 1])
                ksT[2 * halfk] = part[0]
                ksT[2 * halfk + 1] = part[1]
                for mt, p in proj_nat_pair(wsv, hinT, n_list=(halfk,)):
                    dst = vaug_dst(vaug_s[mt], halfk=halfk)
                    src = p[:, halfk * 512:(halfk + 1) * 512]
                    if mt % 2 == 0:
                        nc.scalar.activation(dst, src, AF.Copy, scale=1.0 / WS)
                    else:
                        nc.vector.tensor_scalar(dst, src, 1.0 / WS, None, op0=OP.mult)
                agi = ag_s[halfk]
                emit_kv(ksT, vaug_s, agi, halfk=halfk)
                nc.gpsimd.collective_compute(
                    "AllGather", OP.bypass,
                    replica_groups=[[0, 1], [2, 3], [4, 5], [6, 7]],
                    ins=[agi.opt()], outs=[ag_so[halfk].opt()])
                if halfk == 0:
                    pre_s = prepare_attention(kt_src_s, vt_src_s)

        wsq = load_weight("Wsq")
        wso = load_weight("Wso")

        def q_evict(bc):
            def ev(p, o, mp):
                nc.vector.tensor_mul(o[:], p[:], bc[:])
            return ev

        with nc.named_scope("p4_qself"):
            em_bc = mask_bcast("em", "em", 0.125 * A5 / WS)
            qsT = proj_T_pair(wsq, hinT, "qsT", q_evict(em_bc))

        # =====================================================================
        # Phase 1b: cross K/V matmuls (fills PE while xln/mods run on DVE);
        # the AllGather itself is issued later, after the self-AG halves.
        # =====================================================================
        wck = load_weight("Wck")
        wcv = load_weight("Wcv")
        with nc.named_scope("p1b_crosskv"):
            hn = layernorm(h_sb, BF16, "hn", nc.vector)
            hnT, _ = transpose_act(hn, "hnT")
            kcT = proj_T_pair(wck, hnT, "kcT", kv_evict)
            vaug_c = make_vaug_tiles()
            proj_vaug(wcv, hnT, vaug_c)



        # =====================================================================
        # Phase 4: self attention; cross q
        # =====================================================================
        wcq = load_weight("Wcq")
        wco = load_weight("Wco")
        with nc.named_scope("p5_selfattn"):
            ln2o = [None] * TC

            def ln2_hook(mt, tile_):
                ln2o[mt] = layernorm_tile(tile_, BF16, "ln2o", nc.vector)

            hidden_in = emit_attention(qsT, pre_s, wso,
                                       x_sb, "res1", per_mt_hook=ln2_hook)

        with nc.named_scope("p2_crossag"):
            emit_kv(kcT, vaug_c, ag_in_c)
            nc.gpsimd.collective_compute(
                "AllGather", OP.bypass,
                replica_groups=[[0, 1], [2, 3], [4, 5], [6, 7]],
                ins=[ag_in_c.opt()], outs=[ag_out_c.opt()])
        w1 = load_weight("ffn_w1")
        pre_c = prepare_attention(kt_src_c, vt_src_c)
        with nc.named_scope("p6_ln2q"):
            mk_bc = mask_bcast("mk", "mk", 0.125 * A5 / WS)
            ln2T, _ = transpose_act(ln2o, "ln2T", engines=("s", "s"))
            qcT = proj_T_pair(wcq, ln2T, "qcT", q_evict(mk_bc))

        # =====================================================================
        # Phase 5: cross attention -> hidden_mid
        # =====================================================================
        w2 = load_weight("ffn_w2")
        with nc.named_scope("p7_crossattn"):
            ln3o = [None] * TC

            def ln3_hook(mt, tile_):
                ln3o[mt] = layernorm_tile(tile_, F32, "ln3o", nc.vector,
                                          out_pool=full)

            hidden_mid = emit_attention(qcT, pre_c, wco,
                                        hidden_in, "hmid", per_mt_hook=ln3_hook)

        # =====================================================================
        # Phase 6: FFN + final LN + output (mt-pipelined tail)
        # =====================================================================
        with nc.named_scope("p8_ffn"):
            hoT, _ = transpose_act(ln3o, "hoT", fp32=True, engines=("s", "s"))
            midT_raw = []
            for mp in range(HP):
                p = pspair.tile([128, 1024], F32, name="spair", tag="sp")
                for half in range(2):
                    mo = 2 * mp + half
                    for j in range(4):
                        nc.tensor.matmul(
                            p[:, half * 512:(half + 1) * 512],
                            w1[j][:, :, mo * 128:(mo + 1) * 128],
                            hoT[j][:],
                            start=(j == 0), stop=(j == 3), perf_mode=DR)
                o = e4p.tile([128, 2 * T], E4, name="midT", tag="e4pair")
                if mp % 2 == 0:
                    nc.scalar.activation(o[:], p[:], AF.Relu, scale=1.0 / WS)
                else:
                    nc.vector.tensor_scalar(o[:], p[:], 1.0 / WS, 0.0,
                                            op0=OP.mult, op1=OP.max)
                midT_raw.append(o)
            midT = [o[:].rearrange("p (two f) -> p two f", two=2)
                    for o in midT_raw]

            for mt, p in proj_nat_pair(w2, midT):
                ffres = full.tile([128, H], F32, name="ffres", tag="big")
                nc.vector.scalar_tensor_tensor(
                    ffres[:], p[:], 1.0 / WS,
                    ln3o[mt][:], op0=OP.mult, op1=OP.add)
                lnfo = layernorm_tile(ffres, F32, "lnfo", nc.vector,
                                      out_pool=full)
                nc.vector.tensor_add(lnfo[:], lnfo[:], hidden_mid[mt][:])
                nc.sync.dma_start(out_ext.ap()[mt * 128:(mt + 1) * 128, :],
                                  lnfo[:])


def _bf16(a):
    import ml_dtypes
    return np.asarray(a, np.float32).astype(ml_dtypes.bfloat16)


def _e4(a):
    import ml_dtypes
    return np.clip(np.asarray(a, np.float32), -240, 240).astype(
        ml_dtypes.float8_e4m3)


def _pack_w(w):
    """[K, N] -> [512, 2N] e4m3 pair-packed, prescaled x32."""
    w = np.asarray(w, np.float32) * WS
    K, N = w.shape
    wp = w.reshape(4, 2, 128, N).transpose(0, 2, 1, 3).reshape(512, 2 * N)
    return np.ascontiguousarray(_e4(wp))


def kernel(**inputs):
    global LAST_RESULT
    if "nc" not in _BUILD_CACHE:
        _BUILD_CACHE["nc"] = _build()
    nc = _BUILD_CACHE["nc"]

    x = np.asarray(inputs["x"], np.float32)
    h = np.asarray(inputs["h"], np.float32)
    t = np.asarray(inputs["t"], np.float32)
    em = np.asarray(inputs["extent_mask"], np.int32)
    mk = np.asarray(inputs["mask"], np.int32)

    common = {}
    for nm in W_NAMES + ["ada_w"]:
        common[nm] = _pack_w(inputs[nm])
    common["c_ones"] = _bf16(np.ones((1, T)))
    common["c_ident"] = np.eye(128, dtype=np.float32)
    common["c_identb"] = _bf16(np.eye(128))

    in_maps = []
    for c in range(NC):
        b, half = c // 2, c % 2
        s0 = half * T
        m = dict(common)
        m["x"] = np.ascontiguousarray(_bf16(x[b, s0:s0 + T]))
        m["h"] = np.ascontiguousarray(_bf16(h[b, s0:s0 + T]))
        m["t"] = np.ascontiguousarray(_bf16(t[b, s0:s0 + T]))
        m["em"] = np.ascontiguousarray(em[b, s0:s0 + T].reshape(1, T))
        m["mk"] = np.ascontiguousarray(mk[b, s0:s0 + T].reshape(1, T))
        in_maps.append(m)

    trace = bool(os.environ.get("BASS_TRACE_KERNEL"))
    if trace:
        _install_ntff_hook()
    try:
        res = bass_utils.run_bass_kernel_spmd(
            nc, in_maps, core_ids=list(range(NC)), trace=trace)
    except Exception:
        import time
        time.sleep(20)
        res = bass_utils.run_bass_kernel_spmd(
            nc, in_maps, core_ids=list(range(NC)), trace=trace)
    LAST_RESULT = res

    out = np.empty((B, S, H), np.float32)
    for c in range(NC):
        b, half = c // 2, c % 2
        out[b, half * T:(half + 1) * T] = res.results[c]["out"]
    return out


def _install_ntff_hook():
    import sys, types
    if 'antenv.axon_hooks' in sys.modules:
        return
    mod = types.ModuleType("antenv.axon_hooks")
    mod._hook = None
    def set_axon_ntff_profile_hook(h): mod._hook = h
    def get_axon_ntff_profile_hook(): return mod._hook
    mod.set_axon_ntff_profile_hook = set_axon_ntff_profile_hook
    mod.get_axon_ntff_profile_hook = get_axon_ntff_profile_hook
    sys.modules['antenv.axon_hooks'] = mod
    import antenv
    antenv.axon_hooks = mod
    try:
        from trn_agent_boot.trn_boot import _ntff_profile_via_ctypes
        mod.set_axon_ntff_profile_hook(
            _ntff_profile_via_ctypes('/opt/axon/libaxon_pjrt.so'))
    except Exception:
        pass
